# revision 23
# baseline (speedup 1.0000x reference)
"""NTM Bass kernel builder for TRN2 (all-f32). Per-core: Bl=16 batch, T steps.

Layouts (per core):
  MT  (128m, (b=16, n=128)) f32    memory, m on partitions
  MN  (128n, (b=16, m=128)) f32    memory, n on partitions
  w_state (80=(h,b): p=16h+b, 128n) f32  head weights (h 0-3 read, 4 write)
  colssq (128m, 16b) f32           sum_n Mem^2
  rvT (128m, 64=(b,r): col 4b+r) f32
  outT_all (128cp, (t, ct=4, b=16)) f32

Memory update uses PE rank-1 outer products (w x e, w x a) instead of
partition broadcasts / SBUF-SBUF DMA gathers; the s*a read-vector term is
accumulated straight into the u psum via a k=1 matmul. Output ships as one
contiguous bf16 DMA in raw SBUF layout; the host permutes.
"""
import numpy as np
from contextlib import ExitStack

import concourse.bass as bass
import concourse.tile as tile
from concourse import bacc, mybir

F32 = mybir.dt.float32
BF16 = mybir.dt.bfloat16
AF = mybir.ActivationFunctionType
ALU = mybir.AluOpType

BF_GROUPS = {"mem"}

Bl, N, M, S, R, H = 16, 128, 128, 3, 4, 5
L, LW = 134, 390
NOUT = R * L + LW  # 926
CTRL, INP = 512, 512


def host_prep(x_shard, Wc, bc, Wk, bk, T):
    """Build the per-core input map (numpy, layout prep only)."""
    import ml_dtypes
    f4 = np.float32
    bf = ml_dtypes.bfloat16
    ins = {}
    xT = np.ascontiguousarray(np.transpose(x_shard, (2, 1, 0)).reshape(INP, T * Bl))
    ins["xT"] = xT.astype(bf if "ctrl" in BF_GROUPS else f4)
    Wc1, Wc2 = Wc[:INP], Wc[INP:]
    w1p = np.zeros((128, 16 * 128), f4)
    w2p = np.zeros((128, 16 * 128), f4)
    for kt in range(4):
        for ct in range(4):
            w1p[:, (kt * 4 + ct) * 128:(kt * 4 + ct + 1) * 128] = Wc1[kt * 128:(kt + 1) * 128, ct * 128:(ct + 1) * 128]
            w2p[:, (kt * 4 + ct) * 128:(kt * 4 + ct + 1) * 128] = Wc2[kt * 128:(kt + 1) * 128, ct * 128:(ct + 1) * 128]
    dt_ctrl = bf if "ctrl" in BF_GROUPS else f4
    ins["Wc1p"] = w1p.astype(dt_ctrl)
    ins["Wc2p"] = w2p.astype(dt_ctrl)
    ins["bcrow"] = bc.reshape(1, CTRL).astype(bf)
    perm = np.zeros(NOUT, np.int64)
    pos = 0
    for h in range(5):
        base = h * L if h < 4 else R * L
        perm[pos:pos + 128] = np.arange(base, base + 128)
        pos += 128
    wb = R * L
    perm[pos:pos + 128] = np.arange(wb + L, wb + L + 128); pos += 128          # e
    perm[pos:pos + 128] = np.arange(wb + L + 128, wb + L + 256); pos += 128    # a
    for s_idx in range(6):
        for h in range(5):
            base = h * L if h < 4 else R * L
            perm[pos] = base + 128 + s_idx
            pos += 1
    assert pos == NOUT
    Wkp = np.ascontiguousarray(Wk[:, perm]).astype(f4)
    bkp = np.ascontiguousarray(bk[perm]).astype(f4)
    Wkp[:, 901:906] *= -1.0   # negate g column so sigmoid uses exp(+x)
    bkp[901:906] *= -1.0
    dt_mm2 = bf if "mm2" in BF_GROUPS else f4
    ins["Wkp"] = Wkp.astype(dt_mm2)
    ins["bkrow"] = bkp.reshape(1, NOUT).astype(bf)
    ins["ident_f"] = np.eye(128, dtype=f4)
    dh = np.zeros((5, 16, 80), f4)
    for h in range(5):
        for b in range(16):
            dh[h, b, 16 * h + b] = 1.0
    ins["deltah"] = np.ascontiguousarray(dh.transpose(1, 0, 2).reshape(16, 5 * 80)).astype(bf if "scal" in BF_GROUPS else f4)
    MT0 = np.zeros((128, Bl * 128), f4)
    MN0 = np.zeros((128, Bl * 128), f4)
    for b in range(Bl):
        MT0[:, b * 128 + 64] = 1.0
        MN0[64, b * 128:(b + 1) * 128] = 1.0
    ins["MT0"] = MT0
    ins["MT0bf"] = MT0.astype(bf if "sim" in BF_GROUPS else f4)
    ins["MN0"] = MN0.astype(bf if "uv" in BF_GROUPS else f4)
    ins["colssq0"] = np.ones((128, Bl), f4)
    ins["onesb"] = np.ones((1, 512), bf)
    ins["onesb_f"] = np.ones((1, 512), f4)
    ins["onescol"] = np.ones((128, 1), bf)
    ins["onescol_f"] = np.ones((128, 1), f4)
    blk = np.zeros((16, Bl * 128), f4)
    for b in range(Bl):
        blk[b, b * 128:(b + 1) * 128] = 1.0
    ins["blkones"] = blk
    return ins


def _patch_act_tables():
    """Force Exp/Ln/Square to resolve to the single set containing all three,
    so the scheduler emits one table load instead of thrashing between sets.
    Indices (act_func_set_id) are preserved; only the chooser's view shrinks."""
    import concourse.bacc as _bacc
    if getattr(_bacc, "_ntm_act_patched", False):
        return
    _orig = _bacc.get_activation_tables
    _mb = mybir

    def patched(arch):
        tabs = _orig(arch)
        keep = {_mb.ActivationFunctionType.Exp, _mb.ActivationFunctionType.Ln,
                _mb.ActivationFunctionType.Square}
        out = {}
        for name, funcs in tabs.items():
            if name != "natural_log_exp_and_others":
                funcs = funcs - keep
            out[name] = funcs
        return out

    _bacc.get_activation_tables = patched
    _bacc._ntm_act_patched = True


def build_ntm(T, trace_sim=False, stage="full"):
    _patch_act_tables()
    nc = bacc.Bacc("TRN2", target_bir_lowering=False, debug=False, num_devices=8)
    dt_in = {}

    def din(name, shape, dt=F32):
        dt_in[name] = nc.dram_tensor(name, list(shape), dt, kind="ExternalInput").ap()
        return dt_in[name]

    DT_CTRL = BF16 if "ctrl" in BF_GROUPS else F32
    DT_MM2 = BF16 if "mm2" in BF_GROUPS else F32
    DT_SCAL = BF16 if "scal" in BF_GROUPS else F32
    DT_SIM = BF16 if "sim" in BF_GROUPS else F32
    DT_UV = BF16 if "uv" in BF_GROUPS else F32
    din("xT", (INP, T * Bl), DT_CTRL)
    din("Wc1p", (128, 16 * 128), DT_CTRL)
    din("Wc2p", (128, 16 * 128), DT_CTRL)
    din("bcrow", (1, CTRL), BF16)
    din("Wkp", (INP, NOUT), DT_MM2)
    din("bkrow", (1, NOUT), BF16)
    din("ident_f", (128, 128))
    din("deltah", (16, 5 * 80), DT_SCAL)
    din("MT0", (128, Bl * 128))
    din("MT0bf", (128, Bl * 128), DT_SIM)
    din("MN0", (128, Bl * 128), DT_UV)
    din("colssq0", (128, Bl))
    din("onesb", (1, 512), BF16)
    din("onesb_f", (1, 512))
    din("onescol", (128, 1), BF16)
    din("onescol_f", (128, 1))
    din("blkones", (16, Bl * 128))

    y_d = nc.dram_tensor("y", [128, T * 64], BF16, kind="ExternalOutput").ap()

    with tile.TileContext(nc, trace_sim=trace_sim) as tc:
        with ExitStack() as ctx:
            build_body(nc, tc, ctx, T, dt_in, y_d, stage=stage)
    nc.compile()
    return nc


def build_body(nc, tc, ctx, T, din, y_d, stage="full"):
    DT_CTRL = BF16 if "ctrl" in BF_GROUPS else F32
    DT_MM2 = BF16 if "mm2" in BF_GROUPS else F32
    DT_SCAL = BF16 if "scal" in BF_GROUPS else F32
    DT_SIM = BF16 if "sim" in BF_GROUPS else F32
    DT_UV = BF16 if "uv" in BF_GROUPS else F32
    DT_MEM = BF16 if "mem" in BF_GROUPS else F32
    cpool = ctx.enter_context(tc.tile_pool(name="consts", bufs=1))
    spool = ctx.enter_context(tc.tile_pool(name="state", bufs=1))
    wpool = ctx.enter_context(tc.tile_pool(name="work", bufs=2))
    ppool = ctx.enter_context(tc.tile_pool(name="ps", bufs=1, space="PSUM"))

    # ---------------- load constants/weights ----------------
    TB = T * Bl
    Wc1 = cpool.tile([128, 16 * 128], DT_CTRL, name="Wc1")
    nc.sync.dma_start(Wc1[:], din["Wc1p"])
    Wc2 = cpool.tile([128, 16 * 128], DT_CTRL, name="Wc2")
    nc.sync.dma_start(Wc2[:], din["Wc2p"])
    bcrow = cpool.tile([1, CTRL], BF16, name="bcrow")
    nc.sync.dma_start(bcrow[:], din["bcrow"])
    Wk = cpool.tile([128, 4 * NOUT], DT_MM2, name="Wk")
    for ct in range(4):
        nc.sync.dma_start(Wk[:, ct * NOUT:(ct + 1) * NOUT], din["Wkp"][ct * 128:(ct + 1) * 128, :])
    bkrow = cpool.tile([1, NOUT], BF16, name="bkrow")
    nc.sync.dma_start(bkrow[:], din["bkrow"])
    identf = cpool.tile([128, 128], F32, name="identf")
    nc.sync.dma_start(identf[:], din["ident_f"])
    deltah = cpool.tile([16, 5 * 80], DT_SCAL, name="deltah")
    nc.sync.dma_start(deltah[:], din["deltah"])
    onesb = cpool.tile([1, 512], BF16, name="onesb")
    nc.sync.dma_start(onesb[:], din["onesb"])
    onesb_f = cpool.tile([1, 512], F32, name="onesb_f")
    nc.sync.dma_start(onesb_f[:], din["onesb_f"])
    onescol = cpool.tile([128, 1], BF16, name="onescol")
    nc.sync.dma_start(onescol[:], din["onescol"])
    onescol_f = cpool.tile([128, 1], F32, name="onescol_f")
    nc.sync.dma_start(onescol_f[:], din["onescol_f"])
    ob = {BF16: onesb, F32: onesb_f}
    oc = {BF16: onescol, F32: onescol_f}
    blkones = cpool.tile([16, Bl * 128], F32, name="blkones")
    nc.sync.dma_start(blkones[:], din["blkones"])
    # full xT resident in SBUF: (p, kt*TB + c)
    xsb = cpool.tile([128, 4 * TB], DT_CTRL, name="xsb")
    nc.sync.dma_start(xsb[:].rearrange("p (kt c) -> p kt c", kt=4),
                      din["xT"].rearrange("(kt p) c -> p kt c", kt=4))

    # ---------------- state ----------------
    MT = spool.tile([128, Bl * 128], F32, name="MT_a")
    nc.sync.dma_start(MT[:], din["MT0"])
    MN = spool.tile([128, Bl * 128], DT_UV, name="MN_a")
    nc.sync.dma_start(MN[:], din["MN0"])
    if DT_SIM == BF16:
        MTbf = spool.tile([128, Bl * 128], DT_SIM, name="MTbf_a")
        nc.sync.dma_start(MTbf[:], din["MT0bf"])
    else:
        MTbf = MT
    colssq = spool.tile([128, Bl], F32, name="colssq_a")
    nc.sync.dma_start(colssq[:], din["colssq0"])
    w_state = spool.tile([80, 128], F32, name="w0")
    nc.gpsimd.memset(w_state[:], 0.0)
    rvT = spool.tile([128, 4 * Bl], DT_CTRL, name="rvT0")
    nc.gpsimd.memset(rvT[:], 0.0)
    outT_all = spool.tile([128, T * 64], F32, name="outT_all")
    outT_bf = spool.tile([128, T * 64], BF16, name="outT_bf")


    # ---------------- preamble: xprojT = x @ Wc1 + bc ----------------
    xprojT = spool.tile([128, T * 64], F32, name="xprojT")
    nchunk = (TB + 511) // 512
    for ct in range(4):
        for ch in range(nchunk):
            c0, c1 = ch * 512, min((ch + 1) * 512, TB)
            cw = c1 - c0
            ps_xp = ppool.tile([128, 512], F32, name="ps_xp", tag="mn")
            nc.tensor.matmul(ps_xp[:, :cw], bcrow[0:1, ct * 128:(ct + 1) * 128],
                             onesb[0:1, :cw], start=True, stop=False)
            for kt in range(4):
                nc.tensor.matmul(ps_xp[:, :cw],
                                 Wc1[:, (kt * 4 + ct) * 128:(kt * 4 + ct + 1) * 128],
                                 xsb[:, kt * TB + c0:kt * TB + c1],
                                 start=False, stop=(kt == 3))
            tw = cw // Bl
            src3 = ps_xp[:, :cw].rearrange("p (t b) -> p t b", t=tw)
            dst = bass.AP(xprojT.tensor, ct * 16 + (c0 // Bl) * 64, [[T * 64, 128], [64, tw], [1, 16]])
            if ct % 2 == 0:
                nc.vector.tensor_copy(dst, src3)
            else:
                nc.scalar.copy(dst, src3)

    # ---------------- per-step ----------------
    for t in range(T):
        last = t == T - 1
        b1 = ppool.tile([128, 512], F32, name="b1", tag="b1")
        ps_zT = b1[:, 0:64]
        for ct in range(4):
            for kt in range(4):
                rhs = bass.AP(rvT.tensor, kt, [[4 * Bl, 128], [4, 16]])
                nc.tensor.matmul(ps_zT[:, ct * 16:(ct + 1) * 16],
                                 Wc2[:, (kt * 4 + ct) * 128:(kt * 4 + ct + 1) * 128],
                                 rhs, start=(kt == 0), stop=(kt == 3))
        # ---- tanh: out = 1 - 2/(1+exp(2z)) ----
        z = wpool.tile([128, 64], F32, name="z", tag="z")
        nc.vector.tensor_tensor(z[:], ps_zT, xprojT[:, t * 64:(t + 1) * 64], op=ALU.add)
        Ez = wpool.tile([128, 64], F32, name="Ez", tag="Ez")
        nc.scalar.activation(Ez[:], z[:], AF.Exp, scale=2.0)
        Dz = wpool.tile([128, 64], F32, name="Dz", tag="Dz")
        nc.vector.tensor_scalar(Dz[:], Ez[:], 1.0, None, op0=ALU.add)
        Rz = wpool.tile([128, 64], F32, name="Rz", tag="Rz")
        nc.vector.reciprocal(Rz[:], Dz[:])
        outT = outT_all[:, t * 64:(t + 1) * 64]
        nc.vector.tensor_scalar(outT, Rz[:], -2.0, 1.0, op0=ALU.mult, op1=ALU.add)
        nc.scalar.copy(outT_bf[:, t * 64:(t + 1) * 64], outT)
        if last or stage == "ctrl":
            continue

        # ---- mm2: head instruction psums ----
        b2 = ppool.tile([128, 512], F32, name="b2", tag="b2")
        ps_kq = b2[:, 0:80]
        ps_e = b2[:, 80:96]
        ps_a = b2[:, 96:112]
        ps_scraw = b2[0:16, 112:142]
        ps_ksq = b2[0:16, 144:149]
        nc.tensor.matmul(ps_scraw, onesb[0:1, :16], bkrow[0:1, 896:926], start=True, stop=False)
        for ct in range(4):
            rhs_o = outT_bf[:, t * 64 + ct * 16:t * 64 + (ct + 1) * 16] if DT_MM2 == BF16 else outT[:, ct * 16:(ct + 1) * 16]
            nc.tensor.matmul(ps_scraw, rhs_o,
                             Wk[:, ct * NOUT + 896:ct * NOUT + 926], start=False, stop=(ct == 3))
        for j in range(7):
            tgt = ps_kq[:, j * 16:(j + 1) * 16] if j < 5 else (ps_e if j == 5 else ps_a)
            nc.tensor.matmul(tgt, bkrow[0:1, j * 128:(j + 1) * 128], onesb[0:1, :16],
                             start=True, stop=False)
            for ct in range(4):
                rhs_o = outT_bf[:, t * 64 + ct * 16:t * 64 + (ct + 1) * 16] if DT_MM2 == BF16 else outT[:, ct * 16:(ct + 1) * 16]
                nc.tensor.matmul(tgt, Wk[:, ct * NOUT + j * 128:ct * NOUT + (j + 1) * 128],
                                 rhs_o, start=False, stop=(ct == 3))

        # ---- scalar mini-pipeline in (16, .) ----
        P = wpool.tile([16, 35], F32, name="P", tag="P")
        EXPS = wpool.tile([16, 30], F32, name="EXPS", tag="EXPS")
        nc.scalar.activation(EXPS[:], ps_scraw[:, 0:30], AF.Exp)
        Dg = wpool.tile([16, 5], F32, name="Dg", tag="Dg")
        nc.vector.tensor_scalar(Dg[:], EXPS[:, 5:10], 1.0, None, op0=ALU.add)
        nc.vector.reciprocal(P[:, 5:10], Dg[:])
        nc.vector.tensor_scalar(P[:, 10:15], P[:, 5:10], -1.0, 1.0, op0=ALU.mult, op1=ALU.add)
        ssum = wpool.tile([16, 5], F32, name="ssum", tag="ssum")
        es_v = bass.AP(EXPS.tensor, 10, [[30, 16], [1, 5], [5, 3]])
        nc.vector.tensor_reduce(ssum[:], es_v, axis=mybir.AxisListType.X, op=ALU.add)
        rsum = wpool.tile([16, 5], F32, name="rsum", tag="rsum")
        nc.vector.reciprocal(rsum[:], ssum[:])
        rs_v = bass.AP(rsum.tensor, 0, [[5, 16], [0, 3], [1, 5]])
        nc.vector.tensor_tensor(P[:, 15:30], EXPS[:, 10:25], rs_v, op=ALU.mult)
        k2 = wpool.tile([128, 80], DT_SCAL, name="k2", tag="k2")
        nc.scalar.activation(k2[:], ps_kq, AF.Square)
        for h in range(5):
            nc.tensor.matmul(ps_ksq[:, h:h + 1], k2[:, h * 16:(h + 1) * 16], oc[DT_SCAL][:, 0:1],
                             start=True, stop=True)
        DL = wpool.tile([16, 10], F32, name="DL", tag="DL")
        nc.vector.tensor_scalar(DL[:, 0:5], EXPS[:, 25:30], 1.0, None, op0=ALU.add)
        nc.vector.tensor_scalar(DL[:, 5:10], ps_ksq, 1e-12, None, op0=ALU.max)
        LL = wpool.tile([16, 10], F32, name="LL", tag="LL")
        nc.scalar.activation(LL[:], DL[:], AF.Ln)
        nc.vector.tensor_scalar(P[:, 30:35], LL[:, 0:5], 1.0, None, op0=ALU.add)
        ck = wpool.tile([16, 5], F32, name="ck", tag="ck")
        nc.scalar.activation(ck[:], LL[:, 5:10], AF.Exp, scale=-0.5)
        nc.vector.tensor_tensor(P[:, 0:5], EXPS[:, 0:5], ck[:], op=ALU.mult)
        if DT_SCAL == BF16:
            Pbf = wpool.tile([16, 35], DT_SCAL, name="Pbf", tag="Pbf")
            nc.scalar.copy(Pbf[:], P[:])
        else:
            Pbf = P
        b3 = ppool.tile([128, 512], F32, name="b3", tag="b3")
        ps_scal = b3[0:80, 0:7]
        for h in range(5):
            nc.tensor.matmul(ps_scal, deltah[:, h * 80:(h + 1) * 80], Pbf[:, h::5],
                             start=(h == 0), stop=(h == 4))
        SC = wpool.tile([80, 7], F32, name="SC", tag="SC")
        nc.vector.tensor_copy(SC[:], ps_scal)

        # ---- c_M and q ----
        cmg = wpool.tile([128, 16], F32, name="cmg", tag="cmg")
        nc.vector.tensor_scalar(cmg[:], colssq[:], 1e-12, None, op0=ALU.max)
        Lm = wpool.tile([128, 16], F32, name="Lm", tag="Lm")
        nc.scalar.activation(Lm[:], cmg[:], AF.Ln)
        cM = wpool.tile([128, 16], F32, name="cM", tag="cM")
        nc.scalar.activation(cM[:], Lm[:], AF.Exp, scale=-0.5)
        q = wpool.tile([128, 80], DT_SIM, name="q", tag="q")
        cM_v = bass.AP(cM.tensor, 0, [[16, 128], [0, 5], [1, 16]])
        q3 = q[:].rearrange("p (h b) -> p h b", h=5)
        kq3 = ps_kq.rearrange("p (h b) -> p h b", h=5)
        nc.vector.tensor_tensor(q3, kq3, cM_v, op=ALU.mult)

        # ---- sim ----
        ps_simT = b3[:, 16:96]
        for b in range(Bl):
            nc.tensor.matmul(ps_simT[:, b::16], MTbf[:, b * 128:(b + 1) * 128], q[:, b::16],
                             start=True, stop=True)
        simT = wpool.tile([128, 80], F32, name="simT", tag="simT")
        nc.scalar.copy(simT[:], ps_simT)
        ps_sim = b3[0:80, 96:224]
        nc.tensor.transpose(ps_sim, simT[:], identf[:])
        if stage == "sim":
            continue

        # ---- softmax pipeline (80, 128) ----
        negmax = wpool.tile([80, 1], F32, name="negmax", tag="negmax")
        nc.vector.tensor_reduce(negmax[:], ps_sim, axis=mybir.AxisListType.X, op=ALU.max, negate=True)
        nb = wpool.tile([80, 1], F32, name="nb", tag="nb")
        nc.vector.tensor_tensor(nb[:], negmax[:], SC[:, 0:1], op=ALU.mult)
        EW = wpool.tile([80, 128], F32, name="EW", tag="EW")
        den = wpool.tile([80, 1], F32, name="den", tag="den")
        nc.scalar.activation(EW[:], ps_sim, AF.Exp, bias=nb[:], scale=SC[:, 0:1], accum_out=den[:])
        rden = wpool.tile([80, 1], F32, name="rden", tag="rden")
        nc.vector.reciprocal(rden[:], den[:])
        gd = wpool.tile([80, 1], F32, name="gd", tag="gd")
        nc.vector.tensor_tensor(gd[:], rden[:], SC[:, 1:2], op=ALU.mult)
        BB = wpool.tile([80, 128], F32, name="BB", tag="BB")
        nc.scalar.activation(BB[:], w_state[:], AF.Copy, scale=SC[:, 2:3])
        halo = wpool.tile([80, 130], F32, name="halo", tag="halo")
        nc.vector.scalar_tensor_tensor(halo[:, 1:129], EW[:], gd[:], BB[:], op0=ALU.mult, op1=ALU.add)
        nc.vector.tensor_copy(halo[:, 0:1], halo[:, 128:129])
        nc.vector.tensor_copy(halo[:, 129:130], halo[:, 1:2])
        T1 = wpool.tile([80, 128], F32, name="T1", tag="T1")
        nc.scalar.activation(T1[:], halo[:, 2:130], AF.Copy, scale=SC[:, 5:6])
        T2 = wpool.tile([80, 128], F32, name="T2", tag="T2")
        nc.vector.scalar_tensor_tensor(T2[:], halo[:, 1:129], SC[:, 4:5], T1[:], op0=ALU.mult, op1=ALU.add)
        ws = wpool.tile([80, 128], F32, name="ws", tag="ws")
        nc.vector.scalar_tensor_tensor(ws[:], halo[:, 0:128], SC[:, 3:4], T2[:], op0=ALU.mult, op1=ALU.add)
        Lw = wpool.tile([80, 128], F32, name="Lw", tag="Lw")
        nc.scalar.activation(Lw[:], ws[:], AF.Ln)
        PW = wpool.tile([80, 128], F32, name="PW", tag="PW")
        den2 = wpool.tile([80, 1], F32, name="den2", tag="den2")
        nc.scalar.activation(PW[:], Lw[:], AF.Exp, scale=SC[:, 6:7], accum_out=den2[:])
        rd2 = wpool.tile([80, 1], F32, name="rd2", tag="rd2")
        nc.vector.tensor_scalar(rd2[:], den2[:], 1e-12, None, op0=ALU.add)
        nc.vector.reciprocal(rd2[:], rd2[:])
        w_new = wpool.tile([80, 128], F32, name="w_new", tag="w_new")
        nc.scalar.activation(w_new[:], PW[:], AF.Copy, scale=rd2[:])
        w_state = w_new
        if stage == "softmax":
            continue

        # ---- wT, uvrhs, s ----
        b4 = ppool.tile([128, 512], F32, name="b4", tag="b4")
        ps_wT = b4[:, 128:208]
        nc.tensor.transpose(ps_wT, w_new[:], identf[0:80, 0:80])
        wT = wpool.tile([128, 80], F32, name="wT", tag="wT")
        nc.scalar.copy(wT[:], ps_wT)
        uvrhs = wpool.tile([128, 128], DT_UV, name="uvrhs", tag="uvrhs")
        rw_v = bass.AP(wT.tensor, 0, [[80, 128], [1, 16], [16, 4]])
        ww_v = bass.AP(wT.tensor, 64, [[80, 128], [1, 16], [0, 4]])
        # u-cols: copy rw into uvrhs[:, 8b:8b+4]
        u_dst = bass.AP(uvrhs.tensor, 0, [[128, 128], [8, 16], [1, 4]])
        nc.vector.tensor_copy(u_dst, rw_v)
        # v-cols: rw*ww into uvrhs[:, 8b+4:8b+8]
        v_dst = bass.AP(uvrhs.tensor, 4, [[128, 128], [8, 16], [1, 4]])
        nc.vector.tensor_tensor(v_dst, rw_v, ww_v, op=ALU.mult)
        ps_s = b3[0:64, 224:225]
        rwW_gather = bass.AP(uvrhs.tensor, 4, [[128, 128], [8, 16], [1, 4]])
        rwWc = wpool.tile([128, 64], DT_UV, name="rwWc", tag="rwWc")
        nc.vector.tensor_copy(rwWc[:], rwW_gather)
        nc.tensor.matmul(ps_s, rwWc[:], oc[DT_UV][:, 0:1], start=True, stop=True)
        s_sb = wpool.tile([64, 1], F32, name="s_sb", tag="s_sb")
        nc.vector.tensor_copy(s_sb[:], ps_s)
        ps_srow = b3[0:1, 232:296]
        nc.tensor.transpose(ps_srow, s_sb[:], identf[0:64, 0:64])
        srow = wpool.tile([1, 64], DT_UV, name="srow", tag="srow")
        nc.vector.tensor_copy(srow[:], ps_srow)
        # broadcast s across partitions via PE (ones-column outer product)
        ps_sB = b3[:, 296:360]
        nc.tensor.matmul(ps_sB, ob[DT_UV][0:1, 0:128], srow[0:1, :], start=True, stop=True)

        # ---- e/a copies + row forms ----
        e_f = wpool.tile([128, 16], F32, name="e_f", tag="e_f")
        nc.scalar.copy(e_f[:], ps_e)
        a_f = wpool.tile([128, 16], F32, name="a_f", tag="a_f")
        nc.scalar.copy(a_f[:], ps_a)
        ps_erow = b4[0:16, 336:464]
        nc.tensor.transpose(ps_erow, e_f[:], identf[:])
        ps_arow = b3[0:16, 360:488]
        nc.tensor.transpose(ps_arow, a_f[:], identf[:])
        earows = wpool.tile([16, 128], DT_MEM, name="earows", tag="earows")
        nc.vector.tensor_copy(earows[:], ps_erow)
        arows = wpool.tile([16, 128], DT_MEM, name="arows", tag="arows")
        nc.vector.tensor_copy(arows[:], ps_arow)
        # write-head weights as base-0 rows, then scatter to block-diagonal
        ps_wrow = b4[0:16, 208:336]
        nc.tensor.transpose(ps_wrow, wT[:, 64:80], identf[:])
        wrows = wpool.tile([16, 128], F32, name="wrows", tag="wrows")
        nc.vector.tensor_copy(wrows[:], ps_wrow)

        # ---- u/v MMs + rv assembly ----
        ps_uv = b4[:, 0:128]
        for b in range(Bl):
            nc.tensor.matmul(ps_uv[:, 8 * b:8 * b + 8], MN[:, b * 128:(b + 1) * 128],
                             uvrhs[:, 8 * b:8 * b + 8], start=True, stop=True)
        X1 = wpool.tile([128, 64], F32, name="X1", tag="X1")
        v_v = bass.AP(b4.tensor, 4, [[512, 128], [8, 16], [1, 4]])
        e_v4 = bass.AP(e_f.tensor, 0, [[16, 128], [1, 16], [0, 4]])
        X13 = X1[:].rearrange("p (b r) -> p b r", b=16)
        nc.vector.scalar_tensor_tensor(X13, v_v, -1.0, e_v4, op0=ALU.mult, op1=ALU.mult)
        X2 = wpool.tile([128, 64], F32, name="X2", tag="X2")
        u_v = bass.AP(b4.tensor, 0, [[512, 128], [8, 16], [1, 4]])
        X23 = X2[:].rearrange("p (b r) -> p b r", b=16)
        nc.vector.tensor_tensor(X23, u_v, X13, op=ALU.add)
        X3 = wpool.tile([128, 64], F32, name="X3", tag="X3")
        a_v4 = bass.AP(a_f.tensor, 0, [[16, 128], [1, 16], [0, 4]])
        X33 = X3[:].rearrange("p (b r) -> p b r", b=16)
        nc.vector.tensor_tensor(X33, ps_sB.rearrange("p (b r) -> p b r", b=16), a_v4, op=ALU.mult)
        rvT_new = wpool.tile([128, 64], DT_CTRL, name="rvT_n", tag="rvT_n")
        nc.vector.tensor_tensor(rvT_new[:], X2[:], X3[:], op=ALU.add)
        rvT = rvT_new
        if t == T - 2 or stage == "rv":
            continue

        # ---- memory update via block-diagonal rank-16 products ----
        wbd = wpool.tile([16, Bl * 128], DT_MEM, name="wbd", tag="wbd", bufs=1)
        wrows_bc = bass.AP(wrows.tensor, 0, [[128, 16], [0, 16], [1, 128]])
        nc.vector.tensor_tensor(wbd[:].rearrange("p (b n) -> p b n", b=16),
                                wrows_bc, blkones[:].rearrange("p (b n) -> p b n", b=16),
                                op=ALU.mult)
        MT_new = wpool.tile([128, Bl * 128], F32, name="MT_n", tag="MT_n")
        for g in range(4):
            s0, s1 = g * 512, (g + 1) * 512
            weq = ppool.tile([128, 512], F32, name="we", tag="we")
            waq = ppool.tile([128, 512], F32, name="wa", tag="wa")
            nc.tensor.matmul(weq[:], earows[:], wbd[:, s0:s1], start=True, stop=True)
            nc.tensor.matmul(waq[:], arows[:], wbd[:, s0:s1], start=True, stop=True)
            U = wpool.tile([128, 512], F32, name="Uq", tag="Uq")
            nc.vector.tensor_tensor(U[:], MT[:, s0:s1], weq[:], op=ALU.mult)
            X = wpool.tile([128, 512], F32, name="Xq", tag="Xq")
            nc.vector.scalar_tensor_tensor(X[:], U[:], -1.0, MT[:, s0:s1], op0=ALU.mult, op1=ALU.add)
            nc.vector.tensor_tensor(MT_new[:, s0:s1], X[:], waq[:], op=ALU.add)
        MT = MT_new
        if DT_SIM == BF16:
            MTbf_new = wpool.tile([128, Bl * 128], DT_SIM, name="MTbf_n", tag="MTbf_n")
            for g in range(4):
                s0, s1 = g * 512, (g + 1) * 512
                if g % 2 == 0:
                    nc.scalar.copy(MTbf_new[:, s0:s1], MT[:, s0:s1])
                else:
                    nc.vector.tensor_copy(MTbf_new[:, s0:s1], MT[:, s0:s1])
            MTbf = MTbf_new
        else:
            MTbf = MT
        SQ = wpool.tile([128, Bl * 128], F32, name="SQ", tag="SQ", bufs=1)
        colssq_n = wpool.tile([128, Bl], F32, name="colssq_n", tag="colssq_n")
        for g in range(4):
            s0, s1 = g * 512, (g + 1) * 512
            if g % 2 == 0:
                nc.scalar.activation(SQ[:, s0:s1], MT[:, s0:s1], AF.Square)
            else:
                nc.vector.tensor_tensor(SQ[:, s0:s1], MT[:, s0:s1], MT[:, s0:s1], op=ALU.mult)
            nc.vector.tensor_reduce(colssq_n[:, g * 4:(g + 1) * 4],
                                    SQ[:, s0:s1].rearrange("p (b n) -> p b n", b=4),
                                    axis=mybir.AxisListType.X, op=ALU.add)
        colssq = colssq_n
        MN_new = wpool.tile([128, Bl * 128], DT_UV, name="MN_n", tag="MN_n")
        for g in range(4):
            pm = ppool.tile([128, 512], F32, name="ps_mn", tag=("mn" if g % 2 == 0 else "mn2"))
            for j in range(4):
                b = g * 4 + j
                nc.tensor.transpose(pm[:, j * 128:(j + 1) * 128], MT[:, b * 128:(b + 1) * 128], identf[:])
            if g % 2 == 0:
                nc.vector.tensor_copy(MN_new[:, g * 512:(g + 1) * 512], pm[:])
            else:
                nc.scalar.copy(MN_new[:, g * 512:(g + 1) * 512], pm[:])
        MN = MN_new

    # ---------------- output DMA: one contiguous bf16 transfer ----------------
    nc.sync.dma_start(y_d, outT_bf[:])


# ======================================================================
# SPMD runner: full inputs -> shard over 8 cores -> full output
# ======================================================================
import jax
from jax.sharding import Mesh, PartitionSpec
from jax.experimental.shard_map import shard_map

B_FULL, T_FULL, NCORES = 128, 64, 8

_CACHE = {}


def _get_exec():
    if "exec" in _CACHE:
        return _CACHE["exec"]
    from concourse import bass2jax
    from concourse import mybir as _mb

    nc = build_ntm(T_FULL)
    bass2jax.install_neuronx_cc_hook()

    partition_name = nc.partition_id_tensor.name if nc.partition_id_tensor else None
    in_names, out_names, out_avals, zero_outs = [], [], [], []
    for alloc in nc.m.functions[0].allocations:
        if not isinstance(alloc, _mb.MemoryLocationSet):
            continue
        name = alloc.memorylocations[0].name
        if alloc.kind == "ExternalInput":
            if name != partition_name:
                in_names.append(name)
        elif alloc.kind == "ExternalOutput":
            out_names.append(name)
            shape = tuple(alloc.tensor_shape)
            dtype = _mb.dt.np(alloc.dtype)
            out_avals.append(jax.core.ShapedArray(shape, dtype))
            zero_outs.append(np.zeros(shape, dtype))
    n_params = len(in_names)
    all_names = list(in_names) + list(out_names)
    if partition_name is not None:
        all_names.append(partition_name)

    donate = tuple(range(n_params, n_params + len(out_names)))

    def _body(*args):
        operands = list(args)
        if partition_name is not None:
            operands.append(bass2jax.partition_id_tensor())
        outs = bass2jax._bass_exec_p.bind(
            *operands,
            out_avals=tuple(out_avals),
            in_names=tuple(all_names),
            out_names=tuple(out_names),
            lowering_input_output_aliases=(),
            sim_require_finite=True,
            sim_require_nnan=True,
            nc=nc,
        )
        return tuple(outs)

    devices = jax.devices()[:NCORES]
    mesh = Mesh(np.asarray(devices), ("core",))
    in_specs = (PartitionSpec("core"),) * (n_params + len(out_names))
    out_specs = (PartitionSpec("core"),) * len(out_names)
    fn = jax.jit(
        shard_map(_body, mesh=mesh, in_specs=in_specs, out_specs=out_specs, check_rep=False),
        donate_argnums=donate,
        keep_unused=True,
    )
    ex = dict(nc=nc, fn=fn, in_names=in_names, out_names=out_names,
              zero_outs=zero_outs, out_avals=out_avals, mesh=mesh)
    _CACHE["exec"] = ex
    return ex


def make_concat_inputs(x, Wc, bc, Wk, bk):
    ex = _get_exec()
    per_core = []
    for c in range(NCORES):
        shard = x[c * Bl:(c + 1) * Bl]
        per_core.append(host_prep(shard, Wc, bc, Wk, bk, T_FULL))
    concat = [np.concatenate([per_core[c][nm] for c in range(NCORES)], axis=0)
              for nm in ex["in_names"]]
    return concat


def run_sharded(concat_inputs):
    ex = _get_exec()
    zeros = [np.zeros((NCORES * z.shape[0], *z.shape[1:]), z.dtype) for z in ex["zero_outs"]]
    outs = ex["fn"](*concat_inputs, *zeros)
    return [np.asarray(o) for o in outs]


def kernel(x, Wc, bc, Wk, bk):
    x = np.asarray(x, np.float32)
    Wc = np.asarray(Wc, np.float32)
    bc = np.asarray(bc, np.float32)
    Wk = np.asarray(Wk, np.float32)
    bk = np.asarray(bk, np.float32)
    concat = make_concat_inputs(x, Wc, bc, Wk, bk)
    outs = run_sharded(concat)
    # y raw layout per core: (128cp, t*64 + ct*16 + b) bf16
    raw = np.asarray(outs[0]).astype(np.float32)
    raw = raw.reshape(NCORES, 128, T_FULL, 4, Bl)          # (c, cp, t, ct, b)
    y = raw.transpose(0, 4, 2, 3, 1).reshape(B_FULL, T_FULL, CTRL)
    return y.astype(np.float32)


# revision 29
# speedup vs baseline: 1.0355x; 1.0355x over previous
"""NTM Bass kernel builder for TRN2 (all-f32). Per-core: Bl=16 batch, T steps.

Layouts (per core):
  MT  (128m, (b=16, n=128)) f32    memory, m on partitions
  MN  (128n, (b=16, m=128)) f32    memory, n on partitions
  w_state (80=(h,b): p=16h+b, 128n) f32  head weights (h 0-3 read, 4 write)
  colssq (128m, 16b) f32           sum_n Mem^2
  rvT (128m, 64=(b,r): col 4b+r) f32
  outT_all (128cp, (t, ct=4, b=16)) f32

Memory update uses PE rank-1 outer products (w x e, w x a) instead of
partition broadcasts / SBUF-SBUF DMA gathers; the s*a read-vector term is
accumulated straight into the u psum via a k=1 matmul. Output ships as one
contiguous bf16 DMA in raw SBUF layout; the host permutes.
"""
import numpy as np
from contextlib import ExitStack

import concourse.bass as bass
import concourse.tile as tile
from concourse import bacc, mybir

F32 = mybir.dt.float32
BF16 = mybir.dt.bfloat16
AF = mybir.ActivationFunctionType
ALU = mybir.AluOpType

BF_GROUPS = {"uv", "mem"}

Bl, N, M, S, R, H = 16, 128, 128, 3, 4, 5
L, LW = 134, 390
NOUT = R * L + LW  # 926
CTRL, INP = 512, 512


def host_prep(x_shard, Wc, bc, Wk, bk, T):
    """Build the per-core input map (numpy, layout prep only)."""
    import ml_dtypes
    f4 = np.float32
    bf = ml_dtypes.bfloat16
    ins = {}
    xT = np.ascontiguousarray(np.transpose(x_shard, (2, 1, 0)).reshape(INP, T * Bl))
    ins["xT"] = xT.astype(bf if "ctrl" in BF_GROUPS else f4)
    Wc1, Wc2 = Wc[:INP], Wc[INP:]
    w1p = np.zeros((128, 16 * 128), f4)
    w2p = np.zeros((128, 16 * 128), f4)
    for kt in range(4):
        for ct in range(4):
            w1p[:, (kt * 4 + ct) * 128:(kt * 4 + ct + 1) * 128] = Wc1[kt * 128:(kt + 1) * 128, ct * 128:(ct + 1) * 128]
            w2p[:, (kt * 4 + ct) * 128:(kt * 4 + ct + 1) * 128] = Wc2[kt * 128:(kt + 1) * 128, ct * 128:(ct + 1) * 128]
    dt_ctrl = bf if "ctrl" in BF_GROUPS else f4
    ins["Wc1p"] = w1p.astype(dt_ctrl)
    ins["Wc2p"] = w2p.astype(dt_ctrl)
    ins["bcrow"] = bc.reshape(1, CTRL).astype(bf)
    perm = np.zeros(NOUT, np.int64)
    pos = 0
    for h in range(5):
        base = h * L if h < 4 else R * L
        perm[pos:pos + 128] = np.arange(base, base + 128)
        pos += 128
    wb = R * L
    perm[pos:pos + 128] = np.arange(wb + L, wb + L + 128); pos += 128          # e
    perm[pos:pos + 128] = np.arange(wb + L + 128, wb + L + 256); pos += 128    # a
    for s_idx in range(6):
        for h in range(5):
            base = h * L if h < 4 else R * L
            perm[pos] = base + 128 + s_idx
            pos += 1
    assert pos == NOUT
    Wkp = np.ascontiguousarray(Wk[:, perm]).astype(f4)
    bkp = np.ascontiguousarray(bk[perm]).astype(f4)
    Wkp[:, 901:906] *= -1.0   # negate g column so sigmoid uses exp(+x)
    bkp[901:906] *= -1.0
    Wkhi = Wkp.astype(bf)
    ins["Wkhi"] = Wkhi
    ins["Wklo"] = (Wkp - Wkhi.astype(f4)).astype(bf)
    ins["bkrow"] = bkp.reshape(1, NOUT).astype(bf)
    ins["ident_f"] = np.eye(128, dtype=f4)
    dh = np.zeros((5, 16, 80), f4)
    for h in range(5):
        for b in range(16):
            dh[h, b, 16 * h + b] = 1.0
    ins["deltah"] = np.ascontiguousarray(dh.transpose(1, 0, 2).reshape(16, 5 * 80)).astype(bf if "scal" in BF_GROUPS else f4)
    MT0 = np.zeros((128, Bl * 128), f4)
    MN0 = np.zeros((128, Bl * 128), f4)
    for b in range(Bl):
        MT0[:, b * 128 + 64] = 1.0
        MN0[64, b * 128:(b + 1) * 128] = 1.0
    ins["MT0"] = MT0
    ins["MT0bf"] = MT0.astype(bf if "sim" in BF_GROUPS else f4)
    ins["MN0"] = MN0.astype(bf if "uv" in BF_GROUPS else f4)
    ins["colssq0"] = np.ones((128, Bl), f4)
    ins["onesb"] = np.ones((1, 512), bf)
    ins["onesb_f"] = np.ones((1, 512), f4)
    ins["onescol"] = np.ones((128, 1), bf)
    ins["onescol_f"] = np.ones((128, 1), f4)
    blk = np.zeros((16, Bl * 128), f4)
    for b in range(Bl):
        blk[b, b * 128:(b + 1) * 128] = 1.0
    ins["blkones"] = blk
    return ins


def _patch_act_tables():
    """Force Exp/Ln/Square to resolve to the single set containing all three,
    so the scheduler emits one table load instead of thrashing between sets.
    Indices (act_func_set_id) are preserved; only the chooser's view shrinks."""
    import concourse.bacc as _bacc
    if getattr(_bacc, "_ntm_act_patched", False):
        return
    _orig = _bacc.get_activation_tables
    _mb = mybir

    def patched(arch):
        tabs = _orig(arch)
        keep = {_mb.ActivationFunctionType.Exp, _mb.ActivationFunctionType.Ln,
                _mb.ActivationFunctionType.Square}
        out = {}
        for name, funcs in tabs.items():
            if name != "natural_log_exp_and_others":
                funcs = funcs - keep
            out[name] = funcs
        return out

    _bacc.get_activation_tables = patched
    _bacc._ntm_act_patched = True


def build_ntm(T, trace_sim=False, stage="full"):
    _patch_act_tables()
    nc = bacc.Bacc("TRN2", target_bir_lowering=False, debug=False, num_devices=8)
    dt_in = {}

    def din(name, shape, dt=F32):
        dt_in[name] = nc.dram_tensor(name, list(shape), dt, kind="ExternalInput").ap()
        return dt_in[name]

    DT_CTRL = BF16 if "ctrl" in BF_GROUPS else F32
    DT_MM2 = BF16 if "mm2" in BF_GROUPS else F32
    DT_SCAL = BF16 if "scal" in BF_GROUPS else F32
    DT_SIM = BF16 if "sim" in BF_GROUPS else F32
    DT_UV = BF16 if "uv" in BF_GROUPS else F32
    din("xT", (INP, T * Bl), DT_CTRL)
    din("Wc1p", (128, 16 * 128), DT_CTRL)
    din("Wc2p", (128, 16 * 128), DT_CTRL)
    din("bcrow", (1, CTRL), BF16)
    din("Wkhi", (INP, NOUT), BF16)
    din("Wklo", (INP, NOUT), BF16)
    din("bkrow", (1, NOUT), BF16)
    din("ident_f", (128, 128))
    din("deltah", (16, 5 * 80), DT_SCAL)
    din("MT0", (128, Bl * 128))
    din("MT0bf", (128, Bl * 128), DT_SIM)
    din("MN0", (128, Bl * 128), DT_UV)
    din("colssq0", (128, Bl))
    din("onesb", (1, 512), BF16)
    din("onesb_f", (1, 512))
    din("onescol", (128, 1), BF16)
    din("onescol_f", (128, 1))
    din("blkones", (16, Bl * 128))

    y_d = nc.dram_tensor("y", [128, T * 64], BF16, kind="ExternalOutput").ap()

    with tile.TileContext(nc, trace_sim=trace_sim) as tc:
        with ExitStack() as ctx:
            build_body(nc, tc, ctx, T, dt_in, y_d, stage=stage)
    nc.compile()
    return nc


def build_body(nc, tc, ctx, T, din, y_d, stage="full"):
    DT_CTRL = BF16 if "ctrl" in BF_GROUPS else F32
    DT_MM2 = BF16 if "mm2" in BF_GROUPS else F32
    DT_SCAL = BF16 if "scal" in BF_GROUPS else F32
    DT_SIM = BF16 if "sim" in BF_GROUPS else F32
    DT_UV = BF16 if "uv" in BF_GROUPS else F32
    DT_MEM = BF16 if "mem" in BF_GROUPS else F32
    cpool = ctx.enter_context(tc.tile_pool(name="consts", bufs=1))
    spool = ctx.enter_context(tc.tile_pool(name="state", bufs=1))
    wpool = ctx.enter_context(tc.tile_pool(name="work", bufs=2))
    ppool = ctx.enter_context(tc.tile_pool(name="ps", bufs=1, space="PSUM"))

    # ---------------- load constants/weights ----------------
    TB = T * Bl
    Wc1 = cpool.tile([128, 16 * 128], DT_CTRL, name="Wc1")
    nc.sync.dma_start(Wc1[:], din["Wc1p"])
    Wc2 = cpool.tile([128, 16 * 128], DT_CTRL, name="Wc2")
    nc.sync.dma_start(Wc2[:], din["Wc2p"])
    bcrow = cpool.tile([1, CTRL], BF16, name="bcrow")
    nc.sync.dma_start(bcrow[:], din["bcrow"])
    Wkhi = cpool.tile([128, 4 * NOUT], BF16, name="Wkhi")
    Wklo = cpool.tile([128, 4 * NOUT], BF16, name="Wklo")
    for ct in range(4):
        nc.sync.dma_start(Wkhi[:, ct * NOUT:(ct + 1) * NOUT], din["Wkhi"][ct * 128:(ct + 1) * 128, :])
        nc.sync.dma_start(Wklo[:, ct * NOUT:(ct + 1) * NOUT], din["Wklo"][ct * 128:(ct + 1) * 128, :])
    bkrow = cpool.tile([1, NOUT], BF16, name="bkrow")
    nc.sync.dma_start(bkrow[:], din["bkrow"])
    identf = cpool.tile([128, 128], F32, name="identf")
    nc.sync.dma_start(identf[:], din["ident_f"])
    deltah = cpool.tile([16, 5 * 80], DT_SCAL, name="deltah")
    nc.sync.dma_start(deltah[:], din["deltah"])
    onesb = cpool.tile([1, 512], BF16, name="onesb")
    nc.sync.dma_start(onesb[:], din["onesb"])
    onesb_f = cpool.tile([1, 512], F32, name="onesb_f")
    nc.sync.dma_start(onesb_f[:], din["onesb_f"])
    onescol = cpool.tile([128, 1], BF16, name="onescol")
    nc.sync.dma_start(onescol[:], din["onescol"])
    onescol_f = cpool.tile([128, 1], F32, name="onescol_f")
    nc.sync.dma_start(onescol_f[:], din["onescol_f"])
    ob = {BF16: onesb, F32: onesb_f}
    oc = {BF16: onescol, F32: onescol_f}
    blkones = cpool.tile([16, Bl * 128], F32, name="blkones")
    nc.sync.dma_start(blkones[:], din["blkones"])
    # full xT resident in SBUF: (p, kt*TB + c)
    xsb = cpool.tile([128, 4 * TB], DT_CTRL, name="xsb")
    nc.sync.dma_start(xsb[:].rearrange("p (kt c) -> p kt c", kt=4),
                      din["xT"].rearrange("(kt p) c -> p kt c", kt=4))

    # ---------------- state ----------------
    MT = spool.tile([128, Bl * 128], F32, name="MT_a")
    nc.sync.dma_start(MT[:], din["MT0"])
    MN = spool.tile([128, Bl * 128], DT_UV, name="MN_a")
    nc.sync.dma_start(MN[:], din["MN0"])
    if DT_SIM == BF16:
        MTbf = spool.tile([128, Bl * 128], DT_SIM, name="MTbf_a")
        nc.sync.dma_start(MTbf[:], din["MT0bf"])
    else:
        MTbf = MT
    colssq = spool.tile([128, Bl], F32, name="colssq_a")
    nc.sync.dma_start(colssq[:], din["colssq0"])
    w_state = spool.tile([80, 128], F32, name="w0")
    nc.gpsimd.memset(w_state[:], 0.0)
    rvT = spool.tile([128, 4 * Bl], DT_CTRL, name="rvT0")
    nc.gpsimd.memset(rvT[:], 0.0)
    outT_all = spool.tile([128, T * 64], F32, name="outT_all")
    outT_bf = spool.tile([128, T * 64], BF16, name="outT_bf")


    # ---------------- preamble: xprojT = x @ Wc1 + bc ----------------
    xprojT = spool.tile([128, T * 64], F32, name="xprojT")
    nchunk = (TB + 511) // 512
    for ct in range(4):
        for ch in range(nchunk):
            c0, c1 = ch * 512, min((ch + 1) * 512, TB)
            cw = c1 - c0
            ps_xp = ppool.tile([128, 512], F32, name="ps_xp", tag="we")
            nc.tensor.matmul(ps_xp[:, :cw], bcrow[0:1, ct * 128:(ct + 1) * 128],
                             onesb[0:1, :cw], start=True, stop=False)
            for kt in range(4):
                nc.tensor.matmul(ps_xp[:, :cw],
                                 Wc1[:, (kt * 4 + ct) * 128:(kt * 4 + ct + 1) * 128],
                                 xsb[:, kt * TB + c0:kt * TB + c1],
                                 start=False, stop=(kt == 3))
            tw = cw // Bl
            src3 = ps_xp[:, :cw].rearrange("p (t b) -> p t b", t=tw)
            dst = bass.AP(xprojT.tensor, ct * 16 + (c0 // Bl) * 64, [[T * 64, 128], [64, tw], [1, 16]])
            if ct % 2 == 0:
                nc.vector.tensor_copy(dst, src3)
            else:
                nc.scalar.copy(dst, src3)

    # ---------------- per-step ----------------
    for t in range(T):
        last = t == T - 1
        b1 = ppool.tile([128, 512], F32, name="b1", tag="b1")
        ps_zT = b1[:, 0:64]
        for ct in range(4):
            for kt in range(4):
                rhs = bass.AP(rvT.tensor, kt, [[4 * Bl, 128], [4, 16]])
                nc.tensor.matmul(ps_zT[:, ct * 16:(ct + 1) * 16],
                                 Wc2[:, (kt * 4 + ct) * 128:(kt * 4 + ct + 1) * 128],
                                 rhs, start=(kt == 0), stop=(kt == 3))
        # ---- tanh: out = 1 - 2/(1+exp(2z)) ----
        z = wpool.tile([128, 64], F32, name="z", tag="z")
        nc.vector.tensor_tensor(z[:], ps_zT, xprojT[:, t * 64:(t + 1) * 64], op=ALU.add)
        Ez = wpool.tile([128, 64], F32, name="Ez", tag="Ez")
        nc.scalar.activation(Ez[:], z[:], AF.Exp, scale=2.0)
        Dz = wpool.tile([128, 64], F32, name="Dz", tag="Dz")
        nc.vector.tensor_scalar(Dz[:], Ez[:], 1.0, None, op0=ALU.add)
        Rz = wpool.tile([128, 64], F32, name="Rz", tag="Rz")
        nc.vector.reciprocal(Rz[:], Dz[:])
        outT = outT_all[:, t * 64:(t + 1) * 64]
        nc.vector.tensor_scalar(outT, Rz[:], -2.0, 1.0, op0=ALU.mult, op1=ALU.add)
        nc.scalar.copy(outT_bf[:, t * 64:(t + 1) * 64], outT)
        outT_lo = wpool.tile([128, 64], BF16, name="outT_lo", tag="outT_lo")
        nc.vector.scalar_tensor_tensor(outT_lo[:], outT_bf[:, t * 64:(t + 1) * 64], -1.0, outT,
                                       op0=ALU.mult, op1=ALU.add)
        if last or stage == "ctrl":
            continue

        # ---- mm2 (output-transposed): instr (16b, 926) = out @ Wk + bk ----
        b2 = ppool.tile([128, 512], F32, name="b2", tag="b2")
        ps_kq = b2[:, 0:80]
        ps_e = b2[:, 80:96]
        ps_a = b2[:, 96:112]
        ps_ksq = b2[0:16, 144:149]
        ps_iB = ppool.tile([16, 414], F32, name="ps_iB", tag="mn2")
        ps_iA = ppool.tile([16, 512], F32, name="ps_iA", tag="mn")
        terms = ((outT_bf, Wkhi), (outT_bf, Wklo), (outT_lo, Wkhi))
        nc.tensor.matmul(ps_iB[:], onesb[0:1, :16], bkrow[0:1, 512:926], start=True, stop=False)
        for i, (ox, Wkx) in enumerate(terms):
            for ct in range(4):
                lhs = ox[:, t * 64 + ct * 16:t * 64 + (ct + 1) * 16] if ox is outT_bf else ox[:, ct * 16:(ct + 1) * 16]
                nc.tensor.matmul(ps_iB[:], lhs, Wkx[:, ct * NOUT + 512:ct * NOUT + 926],
                                 start=False, stop=(i == 2 and ct == 3))
        nc.tensor.matmul(ps_iA[:], onesb[0:1, :16], bkrow[0:1, 0:512], start=True, stop=False)
        for i, (ox, Wkx) in enumerate(terms):
            for ct in range(4):
                lhs = ox[:, t * 64 + ct * 16:t * 64 + (ct + 1) * 16] if ox is outT_bf else ox[:, ct * 16:(ct + 1) * 16]
                nc.tensor.matmul(ps_iA[:], lhs, Wkx[:, ct * NOUT:ct * NOUT + 512],
                                 start=False, stop=(i == 2 and ct == 3))
        ps_scraw = ps_iB[:, 384:414]
        I926 = wpool.tile([16, NOUT], F32, name="I926", tag="I926", bufs=1)
        nc.vector.tensor_copy(I926[:, 0:512], ps_iA[:])
        nc.scalar.copy(I926[:, 512:896], ps_iB[:, 0:384])
        for j in range(5):
            nc.tensor.transpose(ps_kq[:, j * 16:(j + 1) * 16], I926[:, j * 128:(j + 1) * 128],
                                identf[0:16, 0:16])
        nc.tensor.transpose(ps_e, I926[:, 640:768], identf[0:16, 0:16])
        nc.tensor.transpose(ps_a, I926[:, 768:896], identf[0:16, 0:16])

        # ---- scalar mini-pipeline in (16, .) ----
        P = wpool.tile([16, 35], F32, name="P", tag="P")
        EXPS = wpool.tile([16, 30], F32, name="EXPS", tag="EXPS")
        nc.scalar.activation(EXPS[:], ps_scraw[:, 0:30], AF.Exp)
        Dg = wpool.tile([16, 5], F32, name="Dg", tag="Dg")
        nc.vector.tensor_scalar(Dg[:], EXPS[:, 5:10], 1.0, None, op0=ALU.add)
        nc.vector.reciprocal(P[:, 5:10], Dg[:])
        nc.vector.tensor_scalar(P[:, 10:15], P[:, 5:10], -1.0, 1.0, op0=ALU.mult, op1=ALU.add)
        ssum = wpool.tile([16, 5], F32, name="ssum", tag="ssum")
        es_v = bass.AP(EXPS.tensor, 10, [[30, 16], [1, 5], [5, 3]])
        nc.vector.tensor_reduce(ssum[:], es_v, axis=mybir.AxisListType.X, op=ALU.add)
        rsum = wpool.tile([16, 5], F32, name="rsum", tag="rsum")
        nc.vector.reciprocal(rsum[:], ssum[:])
        rs_v = bass.AP(rsum.tensor, 0, [[5, 16], [0, 3], [1, 5]])
        nc.vector.tensor_tensor(P[:, 15:30], EXPS[:, 10:25], rs_v, op=ALU.mult)
        k2 = wpool.tile([128, 80], DT_SCAL, name="k2", tag="k2")
        nc.scalar.activation(k2[:], ps_kq, AF.Square)
        for h in range(5):
            nc.tensor.matmul(ps_ksq[:, h:h + 1], k2[:, h * 16:(h + 1) * 16], oc[DT_SCAL][:, 0:1],
                             start=True, stop=True)
        DL = wpool.tile([16, 10], F32, name="DL", tag="DL")
        nc.vector.tensor_scalar(DL[:, 0:5], EXPS[:, 25:30], 1.0, None, op0=ALU.add)
        nc.vector.tensor_scalar(DL[:, 5:10], ps_ksq, 1e-12, None, op0=ALU.max)
        LL = wpool.tile([16, 10], F32, name="LL", tag="LL")
        nc.scalar.activation(LL[:], DL[:], AF.Ln)
        nc.vector.tensor_scalar(P[:, 30:35], LL[:, 0:5], 1.0, None, op0=ALU.add)
        ck = wpool.tile([16, 5], F32, name="ck", tag="ck")
        nc.scalar.activation(ck[:], LL[:, 5:10], AF.Exp, scale=-0.5)
        nc.vector.tensor_tensor(P[:, 0:5], EXPS[:, 0:5], ck[:], op=ALU.mult)
        if DT_SCAL == BF16:
            Pbf = wpool.tile([16, 35], DT_SCAL, name="Pbf", tag="Pbf")
            nc.scalar.copy(Pbf[:], P[:])
        else:
            Pbf = P
        b3 = ppool.tile([128, 512], F32, name="b3", tag="b3")
        ps_scal = b3[0:80, 0:7]
        for h in range(5):
            nc.tensor.matmul(ps_scal, deltah[:, h * 80:(h + 1) * 80], Pbf[:, h::5],
                             start=(h == 0), stop=(h == 4))
        SC = wpool.tile([80, 7], F32, name="SC", tag="SC")
        nc.vector.tensor_copy(SC[:], ps_scal)

        # ---- c_M and q ----
        cmg = wpool.tile([128, 16], F32, name="cmg", tag="cmg")
        nc.vector.tensor_scalar(cmg[:], colssq[:], 1e-12, None, op0=ALU.max)
        Lm = wpool.tile([128, 16], F32, name="Lm", tag="Lm")
        nc.scalar.activation(Lm[:], cmg[:], AF.Ln)
        cM = wpool.tile([128, 16], F32, name="cM", tag="cM")
        nc.scalar.activation(cM[:], Lm[:], AF.Exp, scale=-0.5)
        q = wpool.tile([128, 80], DT_SIM, name="q", tag="q")
        cM_v = bass.AP(cM.tensor, 0, [[16, 128], [0, 5], [1, 16]])
        q3 = q[:].rearrange("p (h b) -> p h b", h=5)
        kq3 = ps_kq.rearrange("p (h b) -> p h b", h=5)
        nc.vector.tensor_tensor(q3, kq3, cM_v, op=ALU.mult)

        # ---- sim ----
        ps_simT = b3[:, 16:96]
        for b in range(Bl):
            nc.tensor.matmul(ps_simT[:, b::16], MTbf[:, b * 128:(b + 1) * 128], q[:, b::16],
                             start=True, stop=True)
        simT = wpool.tile([128, 80], F32, name="simT", tag="simT")
        nc.scalar.copy(simT[:], ps_simT)
        ps_sim = b3[0:80, 96:224]
        nc.tensor.transpose(ps_sim, simT[:], identf[:])
        if stage == "sim":
            continue

        # ---- softmax pipeline (80, 128) ----
        negmax = wpool.tile([80, 1], F32, name="negmax", tag="negmax")
        nc.vector.tensor_reduce(negmax[:], ps_sim, axis=mybir.AxisListType.X, op=ALU.max, negate=True)
        nb = wpool.tile([80, 1], F32, name="nb", tag="nb")
        nc.vector.tensor_tensor(nb[:], negmax[:], SC[:, 0:1], op=ALU.mult)
        EW = wpool.tile([80, 128], F32, name="EW", tag="EW")
        den = wpool.tile([80, 1], F32, name="den", tag="den")
        nc.scalar.activation(EW[:], ps_sim, AF.Exp, bias=nb[:], scale=SC[:, 0:1], accum_out=den[:])
        rden = wpool.tile([80, 1], F32, name="rden", tag="rden")
        nc.vector.reciprocal(rden[:], den[:])
        gd = wpool.tile([80, 1], F32, name="gd", tag="gd")
        nc.vector.tensor_tensor(gd[:], rden[:], SC[:, 1:2], op=ALU.mult)
        BB = wpool.tile([80, 128], F32, name="BB", tag="BB")
        nc.scalar.activation(BB[:], w_state[:], AF.Copy, scale=SC[:, 2:3])
        halo = wpool.tile([80, 130], F32, name="halo", tag="halo")
        nc.vector.scalar_tensor_tensor(halo[:, 1:129], EW[:], gd[:], BB[:], op0=ALU.mult, op1=ALU.add)
        nc.vector.tensor_copy(halo[:, 0:1], halo[:, 128:129])
        nc.vector.tensor_copy(halo[:, 129:130], halo[:, 1:2])
        T1 = wpool.tile([80, 128], F32, name="T1", tag="T1")
        nc.scalar.activation(T1[:], halo[:, 2:130], AF.Copy, scale=SC[:, 5:6])
        T2 = wpool.tile([80, 128], F32, name="T2", tag="T2")
        nc.vector.scalar_tensor_tensor(T2[:], halo[:, 1:129], SC[:, 4:5], T1[:], op0=ALU.mult, op1=ALU.add)
        ws = wpool.tile([80, 128], F32, name="ws", tag="ws")
        nc.vector.scalar_tensor_tensor(ws[:], halo[:, 0:128], SC[:, 3:4], T2[:], op0=ALU.mult, op1=ALU.add)
        Lw = wpool.tile([80, 128], F32, name="Lw", tag="Lw")
        nc.scalar.activation(Lw[:], ws[:], AF.Ln)
        PW = wpool.tile([80, 128], F32, name="PW", tag="PW")
        den2 = wpool.tile([80, 1], F32, name="den2", tag="den2")
        nc.scalar.activation(PW[:], Lw[:], AF.Exp, scale=SC[:, 6:7], accum_out=den2[:])
        rd2 = wpool.tile([80, 1], F32, name="rd2", tag="rd2")
        nc.vector.tensor_scalar(rd2[:], den2[:], 1e-12, None, op0=ALU.add)
        nc.vector.reciprocal(rd2[:], rd2[:])
        w_new = wpool.tile([80, 128], F32, name="w_new", tag="w_new")
        nc.scalar.activation(w_new[:], PW[:], AF.Copy, scale=rd2[:])
        w_state = w_new
        if stage == "softmax":
            continue

        # ---- wT, uvrhs, s ----
        b4 = ppool.tile([128, 512], F32, name="b4", tag="b4")
        ps_wT = b4[:, 128:208]
        nc.tensor.transpose(ps_wT, w_new[:], identf[0:80, 0:80])
        wT = wpool.tile([128, 80], F32, name="wT", tag="wT")
        nc.scalar.copy(wT[:], ps_wT)
        uvrhs = wpool.tile([128, 128], DT_UV, name="uvrhs", tag="uvrhs")
        rw_v = bass.AP(wT.tensor, 0, [[80, 128], [1, 16], [16, 4]])
        ww_v = bass.AP(wT.tensor, 64, [[80, 128], [1, 16], [0, 4]])
        # u-cols: copy rw into uvrhs[:, 8b:8b+4]
        u_dst = bass.AP(uvrhs.tensor, 0, [[128, 128], [8, 16], [1, 4]])
        nc.vector.tensor_copy(u_dst, rw_v)
        # v-cols: rw*ww into uvrhs[:, 8b+4:8b+8]
        v_dst = bass.AP(uvrhs.tensor, 4, [[128, 128], [8, 16], [1, 4]])
        nc.vector.tensor_tensor(v_dst, rw_v, ww_v, op=ALU.mult)
        ps_s = b3[0:64, 224:225]
        rwW_gather = bass.AP(uvrhs.tensor, 4, [[128, 128], [8, 16], [1, 4]])
        rwWc = wpool.tile([128, 64], DT_UV, name="rwWc", tag="rwWc")
        nc.vector.tensor_copy(rwWc[:], rwW_gather)
        nc.tensor.matmul(ps_s, rwWc[:], oc[DT_UV][:, 0:1], start=True, stop=True)
        s_sb = wpool.tile([64, 1], F32, name="s_sb", tag="s_sb")
        nc.vector.tensor_copy(s_sb[:], ps_s)
        ps_srow = b3[0:1, 232:296]
        nc.tensor.transpose(ps_srow, s_sb[:], identf[0:64, 0:64])
        srow = wpool.tile([1, 64], DT_UV, name="srow", tag="srow")
        nc.vector.tensor_copy(srow[:], ps_srow)
        # broadcast s across partitions via PE (ones-column outer product)
        ps_sB = b3[:, 296:360]
        nc.tensor.matmul(ps_sB, ob[DT_UV][0:1, 0:128], srow[0:1, :], start=True, stop=True)

        # ---- e/a copies + row forms ----
        e_f = wpool.tile([128, 16], F32, name="e_f", tag="e_f")
        nc.scalar.copy(e_f[:], ps_e)
        a_f = wpool.tile([128, 16], F32, name="a_f", tag="a_f")
        nc.scalar.copy(a_f[:], ps_a)
        earows = wpool.tile([16, 128], DT_MEM, name="earows", tag="earows")
        nc.vector.tensor_copy(earows[:], I926[:, 640:768])
        arows = wpool.tile([16, 128], DT_MEM, name="arows", tag="arows")
        nc.vector.tensor_copy(arows[:], I926[:, 768:896])
        # write-head weights as base-0 rows, then scatter to block-diagonal
        ps_wrow = b4[0:16, 208:336]
        nc.tensor.transpose(ps_wrow, wT[:, 64:80], identf[:])
        wrows = wpool.tile([16, 128], F32, name="wrows", tag="wrows")
        nc.vector.tensor_copy(wrows[:], ps_wrow)

        # ---- u/v MMs + rv assembly ----
        ps_uv = b4[:, 0:128]
        for b in range(Bl):
            nc.tensor.matmul(ps_uv[:, 8 * b:8 * b + 8], MN[:, b * 128:(b + 1) * 128],
                             uvrhs[:, 8 * b:8 * b + 8], start=True, stop=True)
        X1 = wpool.tile([128, 64], F32, name="X1", tag="X1")
        v_v = bass.AP(b4.tensor, 4, [[512, 128], [8, 16], [1, 4]])
        e_v4 = bass.AP(e_f.tensor, 0, [[16, 128], [1, 16], [0, 4]])
        X13 = X1[:].rearrange("p (b r) -> p b r", b=16)
        nc.vector.scalar_tensor_tensor(X13, v_v, -1.0, e_v4, op0=ALU.mult, op1=ALU.mult)
        X2 = wpool.tile([128, 64], F32, name="X2", tag="X2")
        u_v = bass.AP(b4.tensor, 0, [[512, 128], [8, 16], [1, 4]])
        X23 = X2[:].rearrange("p (b r) -> p b r", b=16)
        nc.vector.tensor_tensor(X23, u_v, X13, op=ALU.add)
        X3 = wpool.tile([128, 64], F32, name="X3", tag="X3")
        a_v4 = bass.AP(a_f.tensor, 0, [[16, 128], [1, 16], [0, 4]])
        X33 = X3[:].rearrange("p (b r) -> p b r", b=16)
        nc.vector.tensor_tensor(X33, ps_sB.rearrange("p (b r) -> p b r", b=16), a_v4, op=ALU.mult)
        rvT_new = wpool.tile([128, 64], DT_CTRL, name="rvT_n", tag="rvT_n")
        nc.vector.tensor_tensor(rvT_new[:], X2[:], X3[:], op=ALU.add)
        rvT = rvT_new
        if t == T - 2 or stage == "rv":
            continue

        # ---- memory update via block-diagonal rank-16 products ----
        wbd = wpool.tile([16, Bl * 128], DT_MEM, name="wbd", tag="wbd", bufs=1)
        wrows_bc = bass.AP(wrows.tensor, 0, [[128, 16], [0, 16], [1, 128]])
        nc.gpsimd.tensor_tensor(wbd[:].rearrange("p (b n) -> p b n", b=16),
                                wrows_bc, blkones[:].rearrange("p (b n) -> p b n", b=16),
                                op=ALU.mult)
        MT_new = wpool.tile([128, Bl * 128], F32, name="MT_n", tag="MT_n")
        for g in range(4):
            s0, s1 = g * 512, (g + 1) * 512
            weq = ppool.tile([128, 512], F32, name="we", tag="we")
            waq = ppool.tile([128, 512], F32, name="wa", tag="wa")
            nc.tensor.matmul(weq[:], earows[:], wbd[:, s0:s1], start=True, stop=True)
            nc.tensor.matmul(waq[:], arows[:], wbd[:, s0:s1], start=True, stop=True)
            U = wpool.tile([128, 512], F32, name="Uq", tag="Uq")
            nc.vector.tensor_tensor(U[:], MT[:, s0:s1], weq[:], op=ALU.mult)
            X = wpool.tile([128, 512], F32, name="Xq", tag="Xq")
            nc.vector.scalar_tensor_tensor(X[:], U[:], -1.0, MT[:, s0:s1], op0=ALU.mult, op1=ALU.add)
            nc.vector.tensor_tensor(MT_new[:, s0:s1], X[:], waq[:], op=ALU.add)
        MT = MT_new
        if DT_SIM == BF16:
            MTbf_new = wpool.tile([128, Bl * 128], DT_SIM, name="MTbf_n", tag="MTbf_n")
            for g in range(4):
                s0, s1 = g * 512, (g + 1) * 512
                if g % 2 == 0:
                    nc.scalar.copy(MTbf_new[:, s0:s1], MT[:, s0:s1])
                else:
                    nc.vector.tensor_copy(MTbf_new[:, s0:s1], MT[:, s0:s1])
            MTbf = MTbf_new
        else:
            MTbf = MT
        SQ = wpool.tile([128, Bl * 128], F32, name="SQ", tag="SQ", bufs=1)
        colssq_n = wpool.tile([128, Bl], F32, name="colssq_n", tag="colssq_n")
        for g in range(4):
            s0, s1 = g * 512, (g + 1) * 512
            if g % 2 == 0:
                nc.scalar.activation(SQ[:, s0:s1], MT[:, s0:s1], AF.Square)
            else:
                nc.vector.tensor_tensor(SQ[:, s0:s1], MT[:, s0:s1], MT[:, s0:s1], op=ALU.mult)
            nc.vector.tensor_reduce(colssq_n[:, g * 4:(g + 1) * 4],
                                    SQ[:, s0:s1].rearrange("p (b n) -> p b n", b=4),
                                    axis=mybir.AxisListType.X, op=ALU.add)
        colssq = colssq_n
        MN_new = wpool.tile([128, Bl * 128], DT_UV, name="MN_n", tag="MN_n")
        for g in range(4):
            pm = ppool.tile([128, 512], F32, name="ps_mn", tag=("we" if g % 2 == 0 else "wa"))
            for j in range(4):
                b = g * 4 + j
                nc.tensor.transpose(pm[:, j * 128:(j + 1) * 128], MT[:, b * 128:(b + 1) * 128], identf[:])
            if g % 2 == 0:
                nc.vector.tensor_copy(MN_new[:, g * 512:(g + 1) * 512], pm[:])
            else:
                nc.scalar.copy(MN_new[:, g * 512:(g + 1) * 512], pm[:])
        MN = MN_new

    # ---------------- output DMA: one contiguous bf16 transfer ----------------
    nc.sync.dma_start(y_d, outT_bf[:])


# ======================================================================
# SPMD runner: full inputs -> shard over 8 cores -> full output
# ======================================================================
import jax
from jax.sharding import Mesh, PartitionSpec
from jax.experimental.shard_map import shard_map

B_FULL, T_FULL, NCORES = 128, 64, 8

_CACHE = {}


def _get_exec():
    if "exec" in _CACHE:
        return _CACHE["exec"]
    from concourse import bass2jax
    from concourse import mybir as _mb

    nc = build_ntm(T_FULL)
    bass2jax.install_neuronx_cc_hook()

    partition_name = nc.partition_id_tensor.name if nc.partition_id_tensor else None
    in_names, out_names, out_avals, zero_outs = [], [], [], []
    for alloc in nc.m.functions[0].allocations:
        if not isinstance(alloc, _mb.MemoryLocationSet):
            continue
        name = alloc.memorylocations[0].name
        if alloc.kind == "ExternalInput":
            if name != partition_name:
                in_names.append(name)
        elif alloc.kind == "ExternalOutput":
            out_names.append(name)
            shape = tuple(alloc.tensor_shape)
            dtype = _mb.dt.np(alloc.dtype)
            out_avals.append(jax.core.ShapedArray(shape, dtype))
            zero_outs.append(np.zeros(shape, dtype))
    n_params = len(in_names)
    all_names = list(in_names) + list(out_names)
    if partition_name is not None:
        all_names.append(partition_name)

    donate = tuple(range(n_params, n_params + len(out_names)))

    def _body(*args):
        operands = list(args)
        if partition_name is not None:
            operands.append(bass2jax.partition_id_tensor())
        outs = bass2jax._bass_exec_p.bind(
            *operands,
            out_avals=tuple(out_avals),
            in_names=tuple(all_names),
            out_names=tuple(out_names),
            lowering_input_output_aliases=(),
            sim_require_finite=True,
            sim_require_nnan=True,
            nc=nc,
        )
        return tuple(outs)

    devices = jax.devices()[:NCORES]
    mesh = Mesh(np.asarray(devices), ("core",))
    in_specs = (PartitionSpec("core"),) * (n_params + len(out_names))
    out_specs = (PartitionSpec("core"),) * len(out_names)
    fn = jax.jit(
        shard_map(_body, mesh=mesh, in_specs=in_specs, out_specs=out_specs, check_rep=False),
        donate_argnums=donate,
        keep_unused=True,
    )
    ex = dict(nc=nc, fn=fn, in_names=in_names, out_names=out_names,
              zero_outs=zero_outs, out_avals=out_avals, mesh=mesh)
    _CACHE["exec"] = ex
    return ex


def make_concat_inputs(x, Wc, bc, Wk, bk):
    ex = _get_exec()
    per_core = []
    for c in range(NCORES):
        shard = x[c * Bl:(c + 1) * Bl]
        per_core.append(host_prep(shard, Wc, bc, Wk, bk, T_FULL))
    concat = [np.concatenate([per_core[c][nm] for c in range(NCORES)], axis=0)
              for nm in ex["in_names"]]
    return concat


def run_sharded(concat_inputs):
    ex = _get_exec()
    zeros = [np.zeros((NCORES * z.shape[0], *z.shape[1:]), z.dtype) for z in ex["zero_outs"]]
    outs = ex["fn"](*concat_inputs, *zeros)
    return [np.asarray(o) for o in outs]


def kernel(x, Wc, bc, Wk, bk):
    x = np.asarray(x, np.float32)
    Wc = np.asarray(Wc, np.float32)
    bc = np.asarray(bc, np.float32)
    Wk = np.asarray(Wk, np.float32)
    bk = np.asarray(bk, np.float32)
    concat = make_concat_inputs(x, Wc, bc, Wk, bk)
    outs = run_sharded(concat)
    # y raw layout per core: (128cp, t*64 + ct*16 + b) bf16
    raw = np.asarray(outs[0]).astype(np.float32)
    raw = raw.reshape(NCORES, 128, T_FULL, 4, Bl)          # (c, cp, t, ct, b)
    y = raw.transpose(0, 4, 2, 3, 1).reshape(B_FULL, T_FULL, CTRL)
    return y.astype(np.float32)


# revision 30
# speedup vs baseline: 1.0370x; 1.0015x over previous
"""NTM Bass kernel builder for TRN2 (all-f32). Per-core: Bl=16 batch, T steps.

Layouts (per core):
  MT  (128m, (b=16, n=128)) f32    memory, m on partitions
  MN  (128n, (b=16, m=128)) f32    memory, n on partitions
  w_state (80=(h,b): p=16h+b, 128n) f32  head weights (h 0-3 read, 4 write)
  colssq (128m, 16b) f32           sum_n Mem^2
  rvT (128m, 64=(b,r): col 4b+r) f32
  outT_all (128cp, (t, ct=4, b=16)) f32

Memory update uses PE rank-1 outer products (w x e, w x a) instead of
partition broadcasts / SBUF-SBUF DMA gathers; the s*a read-vector term is
accumulated straight into the u psum via a k=1 matmul. Output ships as one
contiguous bf16 DMA in raw SBUF layout; the host permutes.
"""
import numpy as np
from contextlib import ExitStack

import concourse.bass as bass
import concourse.tile as tile
from concourse import bacc, mybir

F32 = mybir.dt.float32
BF16 = mybir.dt.bfloat16
AF = mybir.ActivationFunctionType
ALU = mybir.AluOpType

BF_GROUPS = {"uv", "mem"}

Bl, N, M, S, R, H = 16, 128, 128, 3, 4, 5
L, LW = 134, 390
NOUT = R * L + LW  # 926
CTRL, INP = 512, 512


def host_prep(x_shard, Wc, bc, Wk, bk, T):
    """Build the per-core input map (numpy, layout prep only)."""
    import ml_dtypes
    f4 = np.float32
    bf = ml_dtypes.bfloat16
    ins = {}
    xT = np.ascontiguousarray(np.transpose(x_shard, (2, 1, 0)).reshape(INP, T * Bl))
    ins["xT"] = xT.astype(bf if "ctrl" in BF_GROUPS else f4)
    Wc1, Wc2 = Wc[:INP], Wc[INP:]
    w1p = np.zeros((128, 16 * 128), f4)
    w2p = np.zeros((128, 16 * 128), f4)
    for kt in range(4):
        for ct in range(4):
            w1p[:, (kt * 4 + ct) * 128:(kt * 4 + ct + 1) * 128] = Wc1[kt * 128:(kt + 1) * 128, ct * 128:(ct + 1) * 128]
            w2p[:, (kt * 4 + ct) * 128:(kt * 4 + ct + 1) * 128] = Wc2[kt * 128:(kt + 1) * 128, ct * 128:(ct + 1) * 128]
    dt_ctrl = bf if "ctrl" in BF_GROUPS else f4
    ins["Wc1p"] = w1p.astype(dt_ctrl)
    ins["Wc2p"] = w2p.astype(dt_ctrl)
    ins["bcrow"] = bc.reshape(1, CTRL).astype(bf)
    perm = np.zeros(NOUT, np.int64)
    pos = 0
    for h in range(5):
        base = h * L if h < 4 else R * L
        perm[pos:pos + 128] = np.arange(base, base + 128)
        pos += 128
    wb = R * L
    perm[pos:pos + 128] = np.arange(wb + L, wb + L + 128); pos += 128          # e
    perm[pos:pos + 128] = np.arange(wb + L + 128, wb + L + 256); pos += 128    # a
    for s_idx in range(6):
        for h in range(5):
            base = h * L if h < 4 else R * L
            perm[pos] = base + 128 + s_idx
            pos += 1
    assert pos == NOUT
    Wkp = np.ascontiguousarray(Wk[:, perm]).astype(f4)
    bkp = np.ascontiguousarray(bk[perm]).astype(f4)
    Wkp[:, 901:906] *= -1.0   # negate g column so sigmoid uses exp(+x)
    bkp[901:906] *= -1.0
    Wkhi = Wkp.astype(bf)
    ins["Wkhi"] = Wkhi
    ins["Wklo"] = (Wkp - Wkhi.astype(f4)).astype(bf)
    ins["bkrow"] = bkp.reshape(1, NOUT).astype(bf)
    ins["ident_f"] = np.eye(128, dtype=f4)
    dh = np.zeros((5, 16, 80), f4)
    for h in range(5):
        for b in range(16):
            dh[h, b, 16 * h + b] = 1.0
    ins["deltah"] = np.ascontiguousarray(dh.transpose(1, 0, 2).reshape(16, 5 * 80)).astype(bf if "scal" in BF_GROUPS else f4)
    MT0 = np.zeros((128, Bl * 128), f4)
    MN0 = np.zeros((128, Bl * 128), f4)
    for b in range(Bl):
        MT0[:, b * 128 + 64] = 1.0
        MN0[64, b * 128:(b + 1) * 128] = 1.0
    ins["MT0"] = MT0
    ins["MT0bf"] = MT0.astype(bf if "sim" in BF_GROUPS else f4)
    ins["MN0"] = MN0.astype(bf if "uv" in BF_GROUPS else f4)
    ins["colssq0"] = np.ones((128, Bl), f4)
    ins["onesb"] = np.ones((1, 512), bf)
    ins["onesb_f"] = np.ones((1, 512), f4)
    ins["onescol"] = np.ones((128, 1), bf)
    ins["onescol_f"] = np.ones((128, 1), f4)
    blk = np.zeros((16, Bl * 128), f4)
    for b in range(Bl):
        blk[b, b * 128:(b + 1) * 128] = 1.0
    ins["blkones"] = blk
    return ins


def _patch_act_tables():
    """Force Exp/Ln/Square to resolve to the single set containing all three,
    so the scheduler emits one table load instead of thrashing between sets.
    Indices (act_func_set_id) are preserved; only the chooser's view shrinks."""
    import concourse.bacc as _bacc
    if getattr(_bacc, "_ntm_act_patched", False):
        return
    _orig = _bacc.get_activation_tables
    _mb = mybir

    def patched(arch):
        tabs = _orig(arch)
        keep = {_mb.ActivationFunctionType.Exp, _mb.ActivationFunctionType.Ln,
                _mb.ActivationFunctionType.Square}
        out = {}
        for name, funcs in tabs.items():
            if name != "natural_log_exp_and_others":
                funcs = funcs - keep
            out[name] = funcs
        return out

    _bacc.get_activation_tables = patched
    _bacc._ntm_act_patched = True


def build_ntm(T, trace_sim=False, stage="full"):
    _patch_act_tables()
    nc = bacc.Bacc("TRN2", target_bir_lowering=False, debug=False, num_devices=8)
    dt_in = {}

    def din(name, shape, dt=F32):
        dt_in[name] = nc.dram_tensor(name, list(shape), dt, kind="ExternalInput").ap()
        return dt_in[name]

    DT_CTRL = BF16 if "ctrl" in BF_GROUPS else F32
    DT_MM2 = BF16 if "mm2" in BF_GROUPS else F32
    DT_SCAL = BF16 if "scal" in BF_GROUPS else F32
    DT_SIM = BF16 if "sim" in BF_GROUPS else F32
    DT_UV = BF16 if "uv" in BF_GROUPS else F32
    din("xT", (INP, T * Bl), DT_CTRL)
    din("Wc1p", (128, 16 * 128), DT_CTRL)
    din("Wc2p", (128, 16 * 128), DT_CTRL)
    din("bcrow", (1, CTRL), BF16)
    din("Wkhi", (INP, NOUT), BF16)
    din("Wklo", (INP, NOUT), BF16)
    din("bkrow", (1, NOUT), BF16)
    din("ident_f", (128, 128))
    din("deltah", (16, 5 * 80), DT_SCAL)
    din("MT0", (128, Bl * 128))
    din("MT0bf", (128, Bl * 128), DT_SIM)
    din("MN0", (128, Bl * 128), DT_UV)
    din("colssq0", (128, Bl))
    din("onesb", (1, 512), BF16)
    din("onesb_f", (1, 512))
    din("onescol", (128, 1), BF16)
    din("onescol_f", (128, 1))
    din("blkones", (16, Bl * 128))

    y_d = nc.dram_tensor("y", [128, T * 64], BF16, kind="ExternalOutput").ap()

    with tile.TileContext(nc, trace_sim=trace_sim) as tc:
        with ExitStack() as ctx:
            build_body(nc, tc, ctx, T, dt_in, y_d, stage=stage)
    nc.compile()
    return nc


def build_body(nc, tc, ctx, T, din, y_d, stage="full"):
    DT_CTRL = BF16 if "ctrl" in BF_GROUPS else F32
    DT_MM2 = BF16 if "mm2" in BF_GROUPS else F32
    DT_SCAL = BF16 if "scal" in BF_GROUPS else F32
    DT_SIM = BF16 if "sim" in BF_GROUPS else F32
    DT_UV = BF16 if "uv" in BF_GROUPS else F32
    DT_MEM = BF16 if "mem" in BF_GROUPS else F32
    cpool = ctx.enter_context(tc.tile_pool(name="consts", bufs=1))
    spool = ctx.enter_context(tc.tile_pool(name="state", bufs=1))
    wpool = ctx.enter_context(tc.tile_pool(name="work", bufs=2))
    ppool = ctx.enter_context(tc.tile_pool(name="ps", bufs=1, space="PSUM"))

    # ---------------- load constants/weights ----------------
    TB = T * Bl
    Wc1 = cpool.tile([128, 16 * 128], DT_CTRL, name="Wc1")
    nc.sync.dma_start(Wc1[:], din["Wc1p"])
    Wc2 = cpool.tile([128, 16 * 128], DT_CTRL, name="Wc2")
    nc.sync.dma_start(Wc2[:], din["Wc2p"])
    bcrow = cpool.tile([1, CTRL], BF16, name="bcrow")
    nc.sync.dma_start(bcrow[:], din["bcrow"])
    Wkhi = cpool.tile([128, 4 * NOUT], BF16, name="Wkhi")
    Wklo = cpool.tile([128, 4 * NOUT], BF16, name="Wklo")
    for ct in range(4):
        nc.sync.dma_start(Wkhi[:, ct * NOUT:(ct + 1) * NOUT], din["Wkhi"][ct * 128:(ct + 1) * 128, :])
        nc.sync.dma_start(Wklo[:, ct * NOUT:(ct + 1) * NOUT], din["Wklo"][ct * 128:(ct + 1) * 128, :])
    bkrow = cpool.tile([1, NOUT], BF16, name="bkrow")
    nc.sync.dma_start(bkrow[:], din["bkrow"])
    identf = cpool.tile([128, 128], F32, name="identf")
    nc.sync.dma_start(identf[:], din["ident_f"])
    deltah = cpool.tile([16, 5 * 80], DT_SCAL, name="deltah")
    nc.sync.dma_start(deltah[:], din["deltah"])
    onesb = cpool.tile([1, 512], BF16, name="onesb")
    nc.sync.dma_start(onesb[:], din["onesb"])
    onesb_f = cpool.tile([1, 512], F32, name="onesb_f")
    nc.sync.dma_start(onesb_f[:], din["onesb_f"])
    onescol = cpool.tile([128, 1], BF16, name="onescol")
    nc.sync.dma_start(onescol[:], din["onescol"])
    onescol_f = cpool.tile([128, 1], F32, name="onescol_f")
    nc.sync.dma_start(onescol_f[:], din["onescol_f"])
    ob = {BF16: onesb, F32: onesb_f}
    oc = {BF16: onescol, F32: onescol_f}
    blkones = cpool.tile([16, Bl * 128], F32, name="blkones")
    nc.sync.dma_start(blkones[:], din["blkones"])
    # full xT resident in SBUF: (p, kt*TB + c)
    xsb = cpool.tile([128, 4 * TB], DT_CTRL, name="xsb")
    nc.sync.dma_start(xsb[:].rearrange("p (kt c) -> p kt c", kt=4),
                      din["xT"].rearrange("(kt p) c -> p kt c", kt=4))

    # ---------------- state ----------------
    MT = spool.tile([128, Bl * 128], F32, name="MT_a")
    nc.sync.dma_start(MT[:], din["MT0"])
    MN = spool.tile([128, Bl * 128], DT_UV, name="MN_a")
    nc.sync.dma_start(MN[:], din["MN0"])
    if DT_SIM == BF16:
        MTbf = spool.tile([128, Bl * 128], DT_SIM, name="MTbf_a")
        nc.sync.dma_start(MTbf[:], din["MT0bf"])
    else:
        MTbf = MT
    colssq = spool.tile([128, Bl], F32, name="colssq_a")
    nc.sync.dma_start(colssq[:], din["colssq0"])
    w_state = spool.tile([80, 128], F32, name="w0")
    nc.gpsimd.memset(w_state[:], 0.0)
    rvT = spool.tile([128, 4 * Bl], DT_CTRL, name="rvT0")
    nc.gpsimd.memset(rvT[:], 0.0)
    outT_all = spool.tile([128, T * 64], F32, name="outT_all")
    outT_bf = spool.tile([128, T * 64], BF16, name="outT_bf")


    # ---------------- preamble: xprojT = x @ Wc1 + bc ----------------
    xprojT = spool.tile([128, T * 64], F32, name="xprojT")
    nchunk = (TB + 511) // 512
    for ct in range(4):
        for ch in range(nchunk):
            c0, c1 = ch * 512, min((ch + 1) * 512, TB)
            cw = c1 - c0
            ps_xp = ppool.tile([128, 512], F32, name="ps_xp", tag="we")
            nc.tensor.matmul(ps_xp[:, :cw], bcrow[0:1, ct * 128:(ct + 1) * 128],
                             onesb[0:1, :cw], start=True, stop=False)
            for kt in range(4):
                nc.tensor.matmul(ps_xp[:, :cw],
                                 Wc1[:, (kt * 4 + ct) * 128:(kt * 4 + ct + 1) * 128],
                                 xsb[:, kt * TB + c0:kt * TB + c1],
                                 start=False, stop=(kt == 3))
            tw = cw // Bl
            src3 = ps_xp[:, :cw].rearrange("p (t b) -> p t b", t=tw)
            dst = bass.AP(xprojT.tensor, ct * 16 + (c0 // Bl) * 64, [[T * 64, 128], [64, tw], [1, 16]])
            if ct % 2 == 0:
                nc.vector.tensor_copy(dst, src3)
            else:
                nc.scalar.copy(dst, src3)

    # ---------------- per-step ----------------
    for t in range(T):
        last = t == T - 1
        b1 = ppool.tile([128, 512], F32, name="b1", tag="b1")
        ps_zT = b1[:, 0:64]
        for ct in range(4):
            for kt in range(4):
                rhs = bass.AP(rvT.tensor, kt, [[4 * Bl, 128], [4, 16]])
                nc.tensor.matmul(ps_zT[:, ct * 16:(ct + 1) * 16],
                                 Wc2[:, (kt * 4 + ct) * 128:(kt * 4 + ct + 1) * 128],
                                 rhs, start=(kt == 0), stop=(kt == 3))
        # ---- tanh: out = 1 - 2/(1+exp(2z)) ----
        z = wpool.tile([128, 64], F32, name="z", tag="z")
        nc.vector.tensor_tensor(z[:], ps_zT, xprojT[:, t * 64:(t + 1) * 64], op=ALU.add)
        Ez = wpool.tile([128, 64], F32, name="Ez", tag="Ez")
        nc.scalar.activation(Ez[:], z[:], AF.Exp, scale=2.0)
        Dz = wpool.tile([128, 64], F32, name="Dz", tag="Dz")
        nc.vector.tensor_scalar(Dz[:], Ez[:], 1.0, None, op0=ALU.add)
        Rz = wpool.tile([128, 64], F32, name="Rz", tag="Rz")
        nc.vector.reciprocal(Rz[:], Dz[:])
        outT = outT_all[:, t * 64:(t + 1) * 64]
        nc.vector.tensor_scalar(outT, Rz[:], -2.0, 1.0, op0=ALU.mult, op1=ALU.add)
        nc.scalar.copy(outT_bf[:, t * 64:(t + 1) * 64], outT)
        outT_lo = wpool.tile([128, 64], BF16, name="outT_lo", tag="outT_lo")
        nc.vector.scalar_tensor_tensor(outT_lo[:], outT_bf[:, t * 64:(t + 1) * 64], -1.0, outT,
                                       op0=ALU.mult, op1=ALU.add)
        if last or stage == "ctrl":
            continue

        # ---- mm2 (output-transposed): instr (16b, 926) = out @ Wk + bk ----
        b2 = ppool.tile([128, 512], F32, name="b2", tag="b2")
        ps_kq = b2[:, 0:80]
        ps_e = b2[:, 80:96]
        ps_a = b2[:, 96:112]
        ps_ksq = b2[0:16, 144:149]
        ps_iB = ppool.tile([16, 414], F32, name="ps_iB", tag="mn2")
        ps_iA = ppool.tile([16, 512], F32, name="ps_iA", tag="mn")
        terms = ((outT_bf, Wkhi), (outT_bf, Wklo), (outT_lo, Wkhi))
        nc.tensor.matmul(ps_iB[:], onesb[0:1, :16], bkrow[0:1, 512:926], start=True, stop=False)
        for i, (ox, Wkx) in enumerate(terms):
            for ct in range(4):
                lhs = ox[:, t * 64 + ct * 16:t * 64 + (ct + 1) * 16] if ox is outT_bf else ox[:, ct * 16:(ct + 1) * 16]
                nc.tensor.matmul(ps_iB[:], lhs, Wkx[:, ct * NOUT + 512:ct * NOUT + 926],
                                 start=False, stop=(i == 2 and ct == 3))
        nc.tensor.matmul(ps_iA[:], onesb[0:1, :16], bkrow[0:1, 0:512], start=True, stop=False)
        for i, (ox, Wkx) in enumerate(terms):
            for ct in range(4):
                lhs = ox[:, t * 64 + ct * 16:t * 64 + (ct + 1) * 16] if ox is outT_bf else ox[:, ct * 16:(ct + 1) * 16]
                nc.tensor.matmul(ps_iA[:], lhs, Wkx[:, ct * NOUT:ct * NOUT + 512],
                                 start=False, stop=(i == 2 and ct == 3))
        ps_scraw = ps_iB[:, 384:414]
        I926 = wpool.tile([16, NOUT], F32, name="I926", tag="I926", bufs=1)
        nc.vector.tensor_copy(I926[:, 0:512], ps_iA[:])
        nc.scalar.copy(I926[:, 512:896], ps_iB[:, 0:384])
        for j in range(5):
            nc.tensor.transpose(ps_kq[:, j * 16:(j + 1) * 16], I926[:, j * 128:(j + 1) * 128],
                                identf[0:16, 0:16])
        nc.tensor.transpose(ps_e, I926[:, 640:768], identf[0:16, 0:16])
        nc.tensor.transpose(ps_a, I926[:, 768:896], identf[0:16, 0:16])

        # ---- scalar mini-pipeline in (16, .) ----
        P = wpool.tile([16, 35], F32, name="P", tag="P")
        EXPS = wpool.tile([16, 30], F32, name="EXPS", tag="EXPS")
        nc.scalar.activation(EXPS[:], ps_scraw[:, 0:30], AF.Exp)
        Dg = wpool.tile([16, 5], F32, name="Dg", tag="Dg")
        nc.vector.tensor_scalar(Dg[:], EXPS[:, 5:10], 1.0, None, op0=ALU.add)
        nc.vector.reciprocal(P[:, 5:10], Dg[:])
        nc.vector.tensor_scalar(P[:, 10:15], P[:, 5:10], -1.0, 1.0, op0=ALU.mult, op1=ALU.add)
        ssum = wpool.tile([16, 5], F32, name="ssum", tag="ssum")
        es_v = bass.AP(EXPS.tensor, 10, [[30, 16], [1, 5], [5, 3]])
        nc.vector.tensor_reduce(ssum[:], es_v, axis=mybir.AxisListType.X, op=ALU.add)
        rsum = wpool.tile([16, 5], F32, name="rsum", tag="rsum")
        nc.vector.reciprocal(rsum[:], ssum[:])
        rs_v = bass.AP(rsum.tensor, 0, [[5, 16], [0, 3], [1, 5]])
        nc.vector.tensor_tensor(P[:, 15:30], EXPS[:, 10:25], rs_v, op=ALU.mult)
        k2 = wpool.tile([128, 80], DT_SCAL, name="k2", tag="k2")
        nc.scalar.activation(k2[:], ps_kq, AF.Square)
        for h in range(5):
            nc.tensor.matmul(ps_ksq[:, h:h + 1], k2[:, h * 16:(h + 1) * 16], oc[DT_SCAL][:, 0:1],
                             start=True, stop=True)
        DL = wpool.tile([16, 10], F32, name="DL", tag="DL")
        nc.vector.tensor_scalar(DL[:, 0:5], EXPS[:, 25:30], 1.0, None, op0=ALU.add)
        nc.vector.tensor_scalar(DL[:, 5:10], ps_ksq, 1e-12, None, op0=ALU.max)
        LL = wpool.tile([16, 10], F32, name="LL", tag="LL")
        nc.scalar.activation(LL[:], DL[:], AF.Ln)
        nc.vector.tensor_scalar(P[:, 30:35], LL[:, 0:5], 1.0, None, op0=ALU.add)
        ck = wpool.tile([16, 5], F32, name="ck", tag="ck")
        nc.scalar.activation(ck[:], LL[:, 5:10], AF.Exp, scale=-0.5)
        nc.vector.tensor_tensor(P[:, 0:5], EXPS[:, 0:5], ck[:], op=ALU.mult)
        if DT_SCAL == BF16:
            Pbf = wpool.tile([16, 35], DT_SCAL, name="Pbf", tag="Pbf")
            nc.scalar.copy(Pbf[:], P[:])
        else:
            Pbf = P
        b3 = ppool.tile([128, 512], F32, name="b3", tag="b3")
        ps_scal = b3[0:80, 0:7]
        for h in range(5):
            nc.tensor.matmul(ps_scal, deltah[:, h * 80:(h + 1) * 80], Pbf[:, h::5],
                             start=(h == 0), stop=(h == 4))
        SC = wpool.tile([80, 7], F32, name="SC", tag="SC")
        nc.vector.tensor_copy(SC[:], ps_scal)

        # ---- c_M and q ----
        cmg = wpool.tile([128, 16], F32, name="cmg", tag="cmg")
        nc.vector.tensor_scalar(cmg[:], colssq[:], 1e-12, None, op0=ALU.max)
        Lm = wpool.tile([128, 16], F32, name="Lm", tag="Lm")
        nc.scalar.activation(Lm[:], cmg[:], AF.Ln)
        cM = wpool.tile([128, 16], F32, name="cM", tag="cM")
        nc.scalar.activation(cM[:], Lm[:], AF.Exp, scale=-0.5)
        q = wpool.tile([128, 80], DT_SIM, name="q", tag="q")
        cM_v = bass.AP(cM.tensor, 0, [[16, 128], [0, 5], [1, 16]])
        q3 = q[:].rearrange("p (h b) -> p h b", h=5)
        kq3 = ps_kq.rearrange("p (h b) -> p h b", h=5)
        nc.vector.tensor_tensor(q3, kq3, cM_v, op=ALU.mult)

        # ---- sim ----
        ps_simT = b3[:, 16:96]
        for b in range(Bl):
            nc.tensor.matmul(ps_simT[:, b::16], MTbf[:, b * 128:(b + 1) * 128], q[:, b::16],
                             start=True, stop=True)
        simT = wpool.tile([128, 80], F32, name="simT", tag="simT")
        nc.scalar.copy(simT[:], ps_simT)
        ps_sim = b3[0:80, 96:224]
        nc.tensor.transpose(ps_sim, simT[:], identf[:])
        if stage == "sim":
            continue

        # ---- softmax pipeline (80, 128); logits bounded (|sim*beta| < ~5) ----
        EW = wpool.tile([80, 128], F32, name="EW", tag="EW")
        den = wpool.tile([80, 1], F32, name="den", tag="den")
        nc.scalar.activation(EW[:], ps_sim, AF.Exp, scale=SC[:, 0:1], accum_out=den[:])
        rden = wpool.tile([80, 1], F32, name="rden", tag="rden")
        nc.vector.reciprocal(rden[:], den[:])
        gd = wpool.tile([80, 1], F32, name="gd", tag="gd")
        nc.vector.tensor_tensor(gd[:], rden[:], SC[:, 1:2], op=ALU.mult)
        BB = wpool.tile([80, 128], F32, name="BB", tag="BB")
        nc.scalar.activation(BB[:], w_state[:], AF.Copy, scale=SC[:, 2:3])
        halo = wpool.tile([80, 130], F32, name="halo", tag="halo")
        nc.vector.scalar_tensor_tensor(halo[:, 1:129], EW[:], gd[:], BB[:], op0=ALU.mult, op1=ALU.add)
        nc.vector.tensor_copy(halo[:, 0:1], halo[:, 128:129])
        nc.vector.tensor_copy(halo[:, 129:130], halo[:, 1:2])
        T1 = wpool.tile([80, 128], F32, name="T1", tag="T1")
        nc.scalar.activation(T1[:], halo[:, 2:130], AF.Copy, scale=SC[:, 5:6])
        T2 = wpool.tile([80, 128], F32, name="T2", tag="T2")
        nc.vector.scalar_tensor_tensor(T2[:], halo[:, 1:129], SC[:, 4:5], T1[:], op0=ALU.mult, op1=ALU.add)
        ws = wpool.tile([80, 128], F32, name="ws", tag="ws")
        nc.vector.scalar_tensor_tensor(ws[:], halo[:, 0:128], SC[:, 3:4], T2[:], op0=ALU.mult, op1=ALU.add)
        Lw = wpool.tile([80, 128], F32, name="Lw", tag="Lw")
        nc.scalar.activation(Lw[:], ws[:], AF.Ln)
        PW = wpool.tile([80, 128], F32, name="PW", tag="PW")
        den2 = wpool.tile([80, 1], F32, name="den2", tag="den2")
        nc.scalar.activation(PW[:], Lw[:], AF.Exp, scale=SC[:, 6:7], accum_out=den2[:])
        rd2 = wpool.tile([80, 1], F32, name="rd2", tag="rd2")
        nc.vector.reciprocal(rd2[:], den2[:])
        w_new = wpool.tile([80, 128], F32, name="w_new", tag="w_new")
        nc.scalar.activation(w_new[:], PW[:], AF.Copy, scale=rd2[:])
        w_state = w_new
        if stage == "softmax":
            continue

        # ---- wT, uvrhs, s ----
        b4 = ppool.tile([128, 512], F32, name="b4", tag="b4")
        ps_wT = b4[:, 128:208]
        nc.tensor.transpose(ps_wT, w_new[:], identf[0:80, 0:80])
        wT = wpool.tile([128, 80], F32, name="wT", tag="wT")
        nc.scalar.copy(wT[:], ps_wT)
        uvrhs = wpool.tile([128, 128], DT_UV, name="uvrhs", tag="uvrhs")
        rw_v = bass.AP(wT.tensor, 0, [[80, 128], [1, 16], [16, 4]])
        ww_v = bass.AP(wT.tensor, 64, [[80, 128], [1, 16], [0, 4]])
        # u-cols: copy rw into uvrhs[:, 8b:8b+4]
        u_dst = bass.AP(uvrhs.tensor, 0, [[128, 128], [8, 16], [1, 4]])
        nc.vector.tensor_copy(u_dst, rw_v)
        # v-cols: rw*ww into uvrhs[:, 8b+4:8b+8]
        v_dst = bass.AP(uvrhs.tensor, 4, [[128, 128], [8, 16], [1, 4]])
        nc.vector.tensor_tensor(v_dst, rw_v, ww_v, op=ALU.mult)
        ps_s = b3[0:64, 224:225]
        rwW_gather = bass.AP(uvrhs.tensor, 4, [[128, 128], [8, 16], [1, 4]])
        rwWc = wpool.tile([128, 64], DT_UV, name="rwWc", tag="rwWc")
        nc.vector.tensor_copy(rwWc[:], rwW_gather)
        nc.tensor.matmul(ps_s, rwWc[:], oc[DT_UV][:, 0:1], start=True, stop=True)
        s_sb = wpool.tile([64, 1], F32, name="s_sb", tag="s_sb")
        nc.vector.tensor_copy(s_sb[:], ps_s)
        ps_srow = b3[0:1, 232:296]
        nc.tensor.transpose(ps_srow, s_sb[:], identf[0:64, 0:64])
        srow = wpool.tile([1, 64], DT_UV, name="srow", tag="srow")
        nc.vector.tensor_copy(srow[:], ps_srow)
        # broadcast s across partitions via PE (ones-column outer product)
        ps_sB = b3[:, 296:360]
        nc.tensor.matmul(ps_sB, ob[DT_UV][0:1, 0:128], srow[0:1, :], start=True, stop=True)

        # ---- e/a copies + row forms ----
        e_f = wpool.tile([128, 16], F32, name="e_f", tag="e_f")
        nc.scalar.copy(e_f[:], ps_e)
        a_f = wpool.tile([128, 16], F32, name="a_f", tag="a_f")
        nc.scalar.copy(a_f[:], ps_a)
        earows = wpool.tile([16, 128], DT_MEM, name="earows", tag="earows")
        nc.vector.tensor_copy(earows[:], I926[:, 640:768])
        arows = wpool.tile([16, 128], DT_MEM, name="arows", tag="arows")
        nc.vector.tensor_copy(arows[:], I926[:, 768:896])
        # write-head weights as base-0 rows, then scatter to block-diagonal
        ps_wrow = b4[0:16, 208:336]
        nc.tensor.transpose(ps_wrow, wT[:, 64:80], identf[:])
        wrows = wpool.tile([16, 128], F32, name="wrows", tag="wrows")
        nc.vector.tensor_copy(wrows[:], ps_wrow)

        # ---- u/v MMs + rv assembly ----
        ps_uv = b4[:, 0:128]
        for b in range(Bl):
            nc.tensor.matmul(ps_uv[:, 8 * b:8 * b + 8], MN[:, b * 128:(b + 1) * 128],
                             uvrhs[:, 8 * b:8 * b + 8], start=True, stop=True)
        X1 = wpool.tile([128, 64], F32, name="X1", tag="X1")
        v_v = bass.AP(b4.tensor, 4, [[512, 128], [8, 16], [1, 4]])
        e_v4 = bass.AP(e_f.tensor, 0, [[16, 128], [1, 16], [0, 4]])
        X13 = X1[:].rearrange("p (b r) -> p b r", b=16)
        nc.vector.scalar_tensor_tensor(X13, v_v, -1.0, e_v4, op0=ALU.mult, op1=ALU.mult)
        X2 = wpool.tile([128, 64], F32, name="X2", tag="X2")
        u_v = bass.AP(b4.tensor, 0, [[512, 128], [8, 16], [1, 4]])
        X23 = X2[:].rearrange("p (b r) -> p b r", b=16)
        nc.vector.tensor_tensor(X23, u_v, X13, op=ALU.add)
        X3 = wpool.tile([128, 64], F32, name="X3", tag="X3")
        a_v4 = bass.AP(a_f.tensor, 0, [[16, 128], [1, 16], [0, 4]])
        X33 = X3[:].rearrange("p (b r) -> p b r", b=16)
        nc.vector.tensor_tensor(X33, ps_sB.rearrange("p (b r) -> p b r", b=16), a_v4, op=ALU.mult)
        rvT_new = wpool.tile([128, 64], DT_CTRL, name="rvT_n", tag="rvT_n")
        nc.vector.tensor_tensor(rvT_new[:], X2[:], X3[:], op=ALU.add)
        rvT = rvT_new
        if t == T - 2 or stage == "rv":
            continue

        # ---- memory update via block-diagonal rank-16 products ----
        wbd = wpool.tile([16, Bl * 128], DT_MEM, name="wbd", tag="wbd", bufs=1)
        wrows_bc = bass.AP(wrows.tensor, 0, [[128, 16], [0, 16], [1, 128]])
        nc.gpsimd.tensor_tensor(wbd[:].rearrange("p (b n) -> p b n", b=16),
                                wrows_bc, blkones[:].rearrange("p (b n) -> p b n", b=16),
                                op=ALU.mult)
        MT_new = wpool.tile([128, Bl * 128], F32, name="MT_n", tag="MT_n")
        for g in range(4):
            s0, s1 = g * 512, (g + 1) * 512
            weq = ppool.tile([128, 512], F32, name="we", tag="we")
            waq = ppool.tile([128, 512], F32, name="wa", tag="wa")
            nc.tensor.matmul(weq[:], earows[:], wbd[:, s0:s1], start=True, stop=True)
            nc.tensor.matmul(waq[:], arows[:], wbd[:, s0:s1], start=True, stop=True)
            X = wpool.tile([128, 512], F32, name="Xq", tag="Xq")
            nc.vector.tensor_tensor(X[:], MT[:, s0:s1], waq[:], op=ALU.add)
            U = wpool.tile([128, 512], F32, name="Uq", tag="Uq")
            nc.vector.tensor_tensor(U[:], MT[:, s0:s1], weq[:], op=ALU.mult)
            nc.vector.scalar_tensor_tensor(MT_new[:, s0:s1], U[:], -1.0, X[:], op0=ALU.mult, op1=ALU.add)
        MT = MT_new
        if DT_SIM == BF16:
            MTbf_new = wpool.tile([128, Bl * 128], DT_SIM, name="MTbf_n", tag="MTbf_n")
            for g in range(4):
                s0, s1 = g * 512, (g + 1) * 512
                if g % 2 == 0:
                    nc.scalar.copy(MTbf_new[:, s0:s1], MT[:, s0:s1])
                else:
                    nc.vector.tensor_copy(MTbf_new[:, s0:s1], MT[:, s0:s1])
            MTbf = MTbf_new
        else:
            MTbf = MT
        SQ = wpool.tile([128, Bl * 128], F32, name="SQ", tag="SQ", bufs=1)
        colssq_n = wpool.tile([128, Bl], F32, name="colssq_n", tag="colssq_n")
        for g in range(4):
            s0, s1 = g * 512, (g + 1) * 512
            if g % 2 == 0:
                nc.scalar.activation(SQ[:, s0:s1], MT[:, s0:s1], AF.Square)
            else:
                nc.vector.tensor_tensor(SQ[:, s0:s1], MT[:, s0:s1], MT[:, s0:s1], op=ALU.mult)
            nc.vector.tensor_reduce(colssq_n[:, g * 4:(g + 1) * 4],
                                    SQ[:, s0:s1].rearrange("p (b n) -> p b n", b=4),
                                    axis=mybir.AxisListType.X, op=ALU.add)
        colssq = colssq_n
        MN_new = wpool.tile([128, Bl * 128], DT_UV, name="MN_n", tag="MN_n")
        for g in range(4):
            pm = ppool.tile([128, 512], F32, name="ps_mn", tag=("we" if g % 2 == 0 else "wa"))
            for j in range(4):
                b = g * 4 + j
                nc.tensor.transpose(pm[:, j * 128:(j + 1) * 128], MT[:, b * 128:(b + 1) * 128], identf[:])
            if g % 2 == 0:
                nc.vector.tensor_copy(MN_new[:, g * 512:(g + 1) * 512], pm[:])
            else:
                nc.scalar.copy(MN_new[:, g * 512:(g + 1) * 512], pm[:])
        MN = MN_new

    # ---------------- output DMA: one contiguous bf16 transfer ----------------
    nc.sync.dma_start(y_d, outT_bf[:])


# ======================================================================
# SPMD runner: full inputs -> shard over 8 cores -> full output
# ======================================================================
import jax
from jax.sharding import Mesh, PartitionSpec
from jax.experimental.shard_map import shard_map

B_FULL, T_FULL, NCORES = 128, 64, 8

_CACHE = {}


def _get_exec():
    if "exec" in _CACHE:
        return _CACHE["exec"]
    from concourse import bass2jax
    from concourse import mybir as _mb

    nc = build_ntm(T_FULL)
    bass2jax.install_neuronx_cc_hook()

    partition_name = nc.partition_id_tensor.name if nc.partition_id_tensor else None
    in_names, out_names, out_avals, zero_outs = [], [], [], []
    for alloc in nc.m.functions[0].allocations:
        if not isinstance(alloc, _mb.MemoryLocationSet):
            continue
        name = alloc.memorylocations[0].name
        if alloc.kind == "ExternalInput":
            if name != partition_name:
                in_names.append(name)
        elif alloc.kind == "ExternalOutput":
            out_names.append(name)
            shape = tuple(alloc.tensor_shape)
            dtype = _mb.dt.np(alloc.dtype)
            out_avals.append(jax.core.ShapedArray(shape, dtype))
            zero_outs.append(np.zeros(shape, dtype))
    n_params = len(in_names)
    all_names = list(in_names) + list(out_names)
    if partition_name is not None:
        all_names.append(partition_name)

    donate = tuple(range(n_params, n_params + len(out_names)))

    def _body(*args):
        operands = list(args)
        if partition_name is not None:
            operands.append(bass2jax.partition_id_tensor())
        outs = bass2jax._bass_exec_p.bind(
            *operands,
            out_avals=tuple(out_avals),
            in_names=tuple(all_names),
            out_names=tuple(out_names),
            lowering_input_output_aliases=(),
            sim_require_finite=True,
            sim_require_nnan=True,
            nc=nc,
        )
        return tuple(outs)

    devices = jax.devices()[:NCORES]
    mesh = Mesh(np.asarray(devices), ("core",))
    in_specs = (PartitionSpec("core"),) * (n_params + len(out_names))
    out_specs = (PartitionSpec("core"),) * len(out_names)
    fn = jax.jit(
        shard_map(_body, mesh=mesh, in_specs=in_specs, out_specs=out_specs, check_rep=False),
        donate_argnums=donate,
        keep_unused=True,
    )
    ex = dict(nc=nc, fn=fn, in_names=in_names, out_names=out_names,
              zero_outs=zero_outs, out_avals=out_avals, mesh=mesh)
    _CACHE["exec"] = ex
    return ex


def make_concat_inputs(x, Wc, bc, Wk, bk):
    ex = _get_exec()
    per_core = []
    for c in range(NCORES):
        shard = x[c * Bl:(c + 1) * Bl]
        per_core.append(host_prep(shard, Wc, bc, Wk, bk, T_FULL))
    concat = [np.concatenate([per_core[c][nm] for c in range(NCORES)], axis=0)
              for nm in ex["in_names"]]
    return concat


def run_sharded(concat_inputs):
    ex = _get_exec()
    zeros = [np.zeros((NCORES * z.shape[0], *z.shape[1:]), z.dtype) for z in ex["zero_outs"]]
    outs = ex["fn"](*concat_inputs, *zeros)
    return [np.asarray(o) for o in outs]


def kernel(x, Wc, bc, Wk, bk):
    x = np.asarray(x, np.float32)
    Wc = np.asarray(Wc, np.float32)
    bc = np.asarray(bc, np.float32)
    Wk = np.asarray(Wk, np.float32)
    bk = np.asarray(bk, np.float32)
    concat = make_concat_inputs(x, Wc, bc, Wk, bk)
    outs = run_sharded(concat)
    # y raw layout per core: (128cp, t*64 + ct*16 + b) bf16
    raw = np.asarray(outs[0]).astype(np.float32)
    raw = raw.reshape(NCORES, 128, T_FULL, 4, Bl)          # (c, cp, t, ct, b)
    y = raw.transpose(0, 4, 2, 3, 1).reshape(B_FULL, T_FULL, CTRL)
    return y.astype(np.float32)
